# revision 13
# baseline (speedup 1.0000x reference)
"""Trainium2 Bass kernel: 2-layer LSTM (T=80, H=256) + embedding + softmax CE loss.

Strategy: data-parallel over batch (8192 -> 8 cores x 1024).  Everything runs
in a transposed layout: states/gates keep the hidden/gate dim on SBUF
partitions and the batch dim on the free axis, so the recurrent matmuls need
no per-step transposes (stationary = weights, moving = state).

The embedding lookup is reformulated as a one-hot matmul:
    x_t @ W1x  ==  onehot(feat_t) @ (emb @ W1x) = onehot @ E1
E1' = emb @ W1x + b1 + forget_bias_one_hot_fold, so layer-1 needs no bias adds
at all (each sample picks exactly one one-hot row).  The one-hot itself is an
is_equal compare against an iota column, with the feature row replicated
across 80 partitions host-side (pure layout prep).

A logical [256, 1024] tensor is stored "folded" as one SBUF tile [128, 2048]:
hidden unit u lives at (partition u % 128, col-block u // 128).  Gate g's
pre-activations accumulate in one PSUM tile [128, 2048] (4 banks): block ml
holds gate dims 256 g + 128 ml + p.

Final loss: logits computed as Wd.T @ h2 -> [80, B], PE-transposed back to
[B-chunk, 80] so log-sum-exp and the label gather run along the free axis.
"""

import sys

sys.path.insert(0, "/opt/trn_rl_repo")

import numpy as np

import concourse.bass as bass
import concourse.mybir as mybir
import concourse.tile as tile
from concourse import bacc
from concourse.bass_utils import run_bass_kernel_spmd

AF = mybir.ActivationFunctionType
OP = mybir.AluOpType
F32 = mybir.dt.float32
BF16 = mybir.dt.bfloat16
I32 = mybir.dt.int32
FP8 = mybir.dt.float8e4
DT = BF16          # dtype for weights / states / gate activations (matmul operands)
WS = 8.0           # fp8 weight pre-scale (undone by activation scale=1/WS)

P = 128          # partitions
N_CORES = 8
B = 1024         # per-core batch shard
T = 80           # seq len
C = 80           # num classes
E = 8            # emb dim
H = 256          # hidden
G = 4 * H        # gates = 1024
NB = B // 512    # moving-operand chunks of 512 (fp32 max free dim)

GATE_FUNCS = [AF.Sigmoid, AF.Tanh, AF.Sigmoid, AF.Sigmoid]  # i, j, f, o


def build_program(T_steps=T, thin=None, featrep_steps=None):
    # Bacc (not plain Bass): its compile() runs generate_event_semaphores,
    # which splits excess per-instruction sync waits onto InstEventSemaphore
    # nops — walrus only allows one wait on LDWEIGHTS/MATMULT.
    nc = bacc.Bacc("TRN2", target_bir_lowering=False, debug=False,
                   enable_asserts=False, num_devices=N_CORES)

    # ---------------- DRAM I/O ----------------
    featrep = nc.dram_tensor("featrep", [featrep_steps or T_steps, C, B], DT,
                             kind="ExternalInput").ap()
    labelsT = nc.dram_tensor("labelsT", [P, B // P], F32, kind="ExternalInput").ap()
    emb = nc.dram_tensor("emb", [C, E], DT, kind="ExternalInput").ap()
    W1d = nc.dram_tensor("W1", [E + H, G], DT, kind="ExternalInput").ap()
    b1d = nc.dram_tensor("b1", [G], DT, kind="ExternalInput").ap()
    W2d = nc.dram_tensor("W2", [2 * H, G], DT, kind="ExternalInput").ap()
    b2d = nc.dram_tensor("b2", [G], F32, kind="ExternalInput").ap()
    Wdd = nc.dram_tensor("Wd", [H, C], DT, kind="ExternalInput").ap()
    bdd = nc.dram_tensor("bd", [C], F32, kind="ExternalInput").ap()
    ident = nc.dram_tensor("ident80", [C, C], F32, kind="ExternalInput").ap()
    iota80d = nc.dram_tensor("iota80", [C, 1], F32, kind="ExternalInput").ap()
    iotalabd = nc.dram_tensor("iotalab", [P, C], F32, kind="ExternalInput").ap()
    lossd = nc.dram_tensor("loss", [P, B // P], F32, kind="ExternalOutput").ap()

    with tile.TileContext(nc) as tc:
        _emit(nc, tc, featrep, labelsT, emb, W1d, b1d, W2d, b2d, Wdd, bdd,
              ident, iota80d, iotalabd, lossd, T_steps, thin)
    nc.compile()
    return nc


def _emit(nc, tc, featrep, labelsT, emb, W1d, b1d, W2d, b2d, Wdd, bdd,
          ident, iota80d, iotalabd, lossd, T_steps=T, thin=None):
    f32, i32 = F32, I32

    def act(out, in_, func, **kw):
        if thin == "act":
            nc.scalar.activation(out[:, 0:32], in_[:, 0:32], func, **kw)
        else:
            nc.scalar.activation(out, in_, func, **kw)

    def tt(out, a, b_, op):
        if thin == "dve":
            nc.vector.tensor_tensor(out[:, 0:32], a[:, 0:32], b_[:, 0:32], op=op)
        else:
            nc.vector.tensor_tensor(out, a, b_, op=op)

    const = tc.alloc_tile_pool(name="const", bufs=1)

    # ---------------- resident weights/constants ----------------
    # Recurrent weights are stored fp8e4 scaled by WS; the gate activations
    # apply scale=1/WS to undo it.  fp8 + DoubleRow runs the K=256 contraction
    # in ONE matmul at 0.5 cycles/row (2x PE throughput, half the instructions).
    W1h = const.tile([P, 2 * G], DT)          # k-chunk k at cols [G k, G k + G)
    for k in range(2):
        nc.sync.dma_start(out=W1h[:, k * G:(k + 1) * G],
                          in_=W1d[E + P * k: E + P * (k + 1), :])
    W2 = const.tile([P, 4 * G], DT)
    for k in range(4):
        nc.sync.dma_start(out=W2[:, k * G:(k + 1) * G],
                          in_=W2d[P * k: P * (k + 1), :])
    Wd = const.tile([P, 2 * C], DT)
    for k in range(2):
        nc.sync.dma_start(out=Wd[:, k * C:(k + 1) * C],
                          in_=Wdd[P * k: P * (k + 1), :])
    W1h8 = const.tile([P, 2 * G], FP8)
    nc.scalar.activation(W1h8, W1h, AF.Identity, scale=WS)
    W28 = const.tile([P, 4 * G], FP8)
    nc.scalar.activation(W28, W2, AF.Identity, scale=WS)
    Wd8 = const.tile([P, 2 * C], FP8)
    nc.scalar.activation(Wd8, Wd, AF.Identity, scale=WS)
    W1h8k = W1h8.rearrange("p (k g) -> p k g", k=2)
    W28k = W28.rearrange("p (k g) -> p k g", k=4)
    Wd8k = Wd8.rearrange("p (k c) -> p k c", k=2)
    b2c = const.tile([P, G // P], f32)          # col m = b2[128 m : 128 m + 128]
    nc.sync.dma_start(out=b2c, in_=b2d.rearrange("(m p) -> p m", p=P))
    b2rowf = const.tile([1, G], f32)
    nc.sync.dma_start(out=b2rowf, in_=b2d[None, :])
    b2row8 = const.tile([1, G], DT)             # WS * (b2 + forget one-hot)
    nc.vector.tensor_scalar(b2row8, b2rowf, WS, None, op0=OP.mult)
    nc.vector.tensor_scalar_add(b2row8[:, 512:768], b2row8[:, 512:768], WS)
    ones1b = const.tile([1, 512], DT)
    nc.vector.memset(ones1b, 1.0)
    bdc = const.tile([C, 1], f32)
    nc.sync.dma_start(out=bdc, in_=bdd[:, None])
    id80 = const.tile([C, C], f32)
    nc.sync.dma_start(out=id80, in_=ident)
    iota80 = const.tile([C, 1], f32)
    nc.sync.dma_start(out=iota80, in_=iota80d)
    iotalab = const.tile([P, C], f32)
    nc.sync.dma_start(out=iotalab, in_=iotalabd)
    labT = const.tile([P, B // P], f32)
    nc.sync.dma_start(out=labT, in_=labelsT)

    # forget-gate bias for layer 2: +1.0 on gate dims [512, 768) = cols 4,5
    nc.vector.tensor_scalar_add(b2c[:, 4:6], b2c[:, 4:6], 1.0)

    # ---------------- E1' = emb @ W1x + b1 (+1 on f-range) ----------------
    startup = tc.alloc_tile_pool(name="startup", bufs=1)
    embT = startup.tile([E, C], DT)
    nc.sync.dma_start(out=embT, in_=emb.rearrange("c e -> e c"))
    W1x = startup.tile([E, G], DT)
    nc.sync.dma_start(out=W1x, in_=W1d[0:E, :])
    b1row = startup.tile([1, G], DT)
    nc.sync.dma_start(out=b1row, in_=b1d[None, :])
    ones1 = startup.tile([1, C], DT)
    nc.vector.memset(ones1, 1.0)

    pstart = tc.alloc_tile_pool(name="pstart", bufs=1, space="PSUM")
    e1ps = pstart.tile([C, G], f32)
    for n in range(2):
        s = slice(512 * n, 512 * (n + 1))
        nc.tensor.matmul(e1ps[:, s], embT, W1x[:, s], start=True, stop=False)
        nc.tensor.matmul(e1ps[:, s], ones1, b1row[:, s], start=False, stop=True)
    E1 = const.tile([C, G], DT)
    nc.scalar.activation(E1, e1ps, AF.Identity, scale=WS)  # pre-scaled like W1h8
    nc.vector.tensor_scalar_add(E1[:, 512:768], E1[:, 512:768], WS)  # forget bias
    pstart.release()
    startup.release()

    # ---------------- pools for the recurrent loop ----------------
    states = tc.alloc_tile_pool(name="states", bufs=2)
    gates = tc.alloc_tile_pool(name="gates", bufs=2)
    pgate = tc.alloc_tile_pool(name="pgate", bufs=2, space="PSUM")
    feats = tc.alloc_tile_pool(name="feats", bufs=3)

    h1 = c1 = h2 = c2 = None
    SI = 1.0 / WS

    def l1_block(t, oh, h1_in):
        # One PSUM tile + ONE activation per gate ([P, 2B] = 4 banks).
        # Emitted in gate PAIRS with all (dependency-free) E1 matmuls first,
        # so the in-order PE queue can pre-run them while h1 is still being
        # produced, instead of stalling at the first DoubleRow matmul.
        h1k = h1_in.rearrange("p (k b) -> p k b", k=2) if t > 0 else None
        e1_only = (t == 0 or thin == "pe")
        sg1 = []
        for gp in range(2):
            pair = (2 * gp, 2 * gp + 1)
            ps_t = {}
            for g in pair:
                ps_t[g] = pgate.tile([P, 2 * B], f32, tag="g", name=f"ps1_{t}_{g}")
                for ml in range(2):
                    m = 2 * g + ml
                    for n in range(NB):
                        dst = ps_t[g][:, B * ml + 512 * n: B * ml + 512 * (n + 1)]
                        rhs_oh = oh[:, 512 * n: 512 * (n + 1)]
                        nc.tensor.matmul(dst, E1[:, P * m: P * (m + 1)], rhs_oh,
                                         start=True, stop=e1_only)
            for g in pair:
                if not e1_only:
                    for ml in range(2):
                        m = 2 * g + ml
                        for n in range(NB):
                            dst = ps_t[g][:, B * ml + 512 * n: B * ml + 512 * (n + 1)]
                            nc.tensor.matmul(
                                dst,
                                W1h8k[:, :, P * m: P * (m + 1)],
                                h1k[:, :, 512 * n: 512 * (n + 1)],
                                start=False, stop=True,
                                perf_mode=mybir.MatmulPerfMode.DoubleRow)
                sg = gates.tile([P, 2 * B], DT, tag=f"sg_{g}", name=f"sg1_{g}")
                act(sg, ps_t[g], GATE_FUNCS[g], scale=SI)
                sg1.append(sg)
        return sg1

    def l2_block(t, h1_in, h2_in):
        # Bias enters the PSUM group as a K=1 matmul (b2row8.T @ ones) so the
        # whole [P, 2B] gate tile takes ONE activation; the bias matmuls are
        # dependency-free, giving the in-order PE queue work to pre-run.
        h1k = h1_in.rearrange("p (k b) -> p k b", k=2)
        h2k = h2_in.rearrange("p (k b) -> p k b", k=2) if t > 0 else None
        kks = [0] if (t == 0 or thin == "pe") else [0, 1]
        sg2 = []
        for g in range(4):
            sg = gates.tile([P, 2 * B], DT, tag=f"sg_{g}", name=f"sg2_{g}")
            ps = pgate.tile([P, 2 * B], f32, tag="g", name=f"ps2_{t}_{g}")
            for ml in range(2):
                m = 2 * g + ml
                for n in range(NB):
                    dst = ps[:, B * ml + 512 * n: B * ml + 512 * (n + 1)]
                    nc.tensor.matmul(dst, b2row8[:, P * m: P * (m + 1)],
                                     ones1b, start=True, stop=False)
            for ml in range(2):
                m = 2 * g + ml
                for n in range(NB):
                    dst = ps[:, B * ml + 512 * n: B * ml + 512 * (n + 1)]
                    for ki, kk in enumerate(kks):
                        hsrck = h1k if kk == 0 else h2k
                        nc.tensor.matmul(
                            dst,
                            W28k[:, 2 * kk: 2 * kk + 2, P * m: P * (m + 1)],
                            hsrck[:, :, 512 * n: 512 * (n + 1)],
                            start=False, stop=(ki == len(kks) - 1),
                            perf_mode=mybir.MatmulPerfMode.DoubleRow)
            act(sg, ps, GATE_FUNCS[g], scale=SI)
            sg2.append(sg)
        return sg2

    def cell(t, sgates, c_in, ctag, htag, thname):
        si, sj, sf, so = sgates
        tt(si, si, sj, OP.mult)                  # si <- sigmoid(i)*tanh(j)
        cn = states.tile([P, 2 * B], DT, tag=ctag, name=ctag)
        if t == 0:
            nc.vector.tensor_copy(cn, si)
        else:
            tt(sf, c_in, sf, OP.mult)            # sf <- c * sigmoid(f+1)
            tt(cn, sf, si, OP.add)
        th = gates.tile([P, 2 * B], DT, tag="th", name=thname, bufs=2)
        act(th, cn, AF.Tanh)
        hn = states.tile([P, 2 * B], FP8, tag=htag, name=htag)
        tt(hn, th, so, OP.mult)
        return cn, hn

    # Software pipeline: L2 runs one step behind L1.  The L1 cell (which
    # produces h1(t), the recurrent critical path) is emitted BEFORE the L2
    # block so its tanh isn't queued behind 8 unrelated L2 activations on
    # the in-order ACT engine.
    sg2_pend = None
    n_feat = featrep.shape[0]
    for t in range(T_steps):
        featbc = feats.tile([C, B], DT, tag="featbc")
        nc.sync.dma_start(out=featbc, in_=featrep[t % n_feat])
        oh = feats.tile([C, B], DT, tag="oh")
        nc.vector.tensor_scalar(oh, featbc, iota80[:, 0:1], None, op0=OP.is_equal)

        h1_prev = h1
        sg1 = l1_block(t, oh, h1_prev)
        c1, h1 = cell(t, sg1, c1, "c1", "h1", "th1")
        if t > 0:
            sg2_pend = l2_block(t - 1, h1_prev, h2)
            c2, h2 = cell(t - 1, sg2_pend, c2, "c2", "h2", "th2")

    # drain the pipeline: L2 for the final step
    sg2_pend = l2_block(T_steps - 1, h1, h2)
    c2, h2 = cell(T_steps - 1, sg2_pend, c2, "c2", "h2", "th2")

    feats.release()
    pgate.release()

    # ---------------- loss ----------------
    ploss = tc.alloc_tile_pool(name="ploss", bufs=1, space="PSUM")
    lpool = tc.alloc_tile_pool(name="lpool", bufs=2)

    lps = ploss.tile([C, B], f32, tag="logits")
    h2k = h2.rearrange("p (k b) -> p k b", k=2)
    for n in range(NB):
        nc.tensor.matmul(
            lps[:, 512 * n: 512 * (n + 1)],
            Wd8k,
            h2k[:, :, 512 * n: 512 * (n + 1)],
            start=True, stop=True,
            perf_mode=mybir.MatmulPerfMode.DoubleRow)
    logits = lpool.tile([C, B], f32, tag="logits_sb", bufs=1)
    nc.scalar.activation(logits, lps, AF.Identity, bias=bdc[:, 0:1], scale=SI)

    loss_sb = lpool.tile([P, B // P], f32, tag="loss_sb", bufs=1)
    # per-chunk sum-exps / label logits gathered as COLUMNS of shared tiles so
    # the log and the final subtract are single ops (one Exp->Ln table switch)
    sumexp_all = lpool.tile([P, B // P], f32, tag="sumexp_all", bufs=1)
    lablog_all = lpool.tile([P, B // P], f32, tag="lablog_all", bufs=1)
    for cb in range(B // P):
        lt = ploss.tile([P, C], f32, tag="lt", bufs=2, name=f"lt_{cb}")
        nc.tensor.transpose(lt, logits[:, P * cb: P * (cb + 1)], id80)
        ohl = lpool.tile([P, C], f32, tag="ohl", name=f"ohl_{cb}")
        nc.vector.tensor_scalar(ohl, iotalab, labT[:, cb:cb + 1], None,
                                op0=OP.is_equal)
        scr1 = lpool.tile([P, C], f32, tag="scr1", name=f"scr1_{cb}")
        nc.vector.scalar_tensor_tensor(scr1, lt, 1.0, ohl,
                                       op0=OP.mult, op1=OP.mult,
                                       accum_out=lablog_all[:, cb:cb + 1])
        scr2 = lpool.tile([P, C], f32, tag="scr2", name=f"scr2_{cb}")
        nc.scalar.activation(scr2, lt, AF.Exp,
                             accum_out=sumexp_all[:, cb:cb + 1])
    lse = lpool.tile([P, B // P], f32, tag="lse", bufs=1)
    nc.scalar.activation(lse, sumexp_all, AF.Ln)
    nc.vector.tensor_sub(loss_sb, lse, lablog_all)
    nc.sync.dma_start(out=lossd, in_=loss_sb)
    lpool.release()
    ploss.release()
    gates.release()
    states.release()
    const.release()


# ---------------------------------------------------------------------------
# host side
# ---------------------------------------------------------------------------
_CACHE = {}


def _get_program():
    if "nc" not in _CACHE:
        _CACHE["nc"] = build_program()
    return _CACHE["nc"]


def make_in_maps(features, labels, embedding, W1, b1, W2, b2, Wd, bd):
    """Shard the full inputs into 8 per-core input maps."""
    features = np.asarray(features, dtype=np.int32)
    labels = np.asarray(labels, dtype=np.int32)
    import ml_dtypes
    wdt = ml_dtypes.bfloat16 if DT == BF16 else np.float32
    shared = {
        "emb": np.ascontiguousarray(np.asarray(embedding, np.float32).astype(wdt)),
        "W1": np.ascontiguousarray(np.asarray(W1, np.float32).astype(wdt)),
        "b1": np.ascontiguousarray(np.asarray(b1, np.float32).astype(wdt)),
        "W2": np.ascontiguousarray(np.asarray(W2, np.float32).astype(wdt)),
        "b2": np.ascontiguousarray(b2, dtype=np.float32),
        "Wd": np.ascontiguousarray(np.asarray(Wd, np.float32).astype(wdt)),
        "bd": np.ascontiguousarray(bd, dtype=np.float32),
        "ident80": np.eye(C, dtype=np.float32),
        "iota80": np.arange(C, dtype=np.float32).reshape(C, 1),
        "iotalab": np.ascontiguousarray(
            np.broadcast_to(np.arange(C, dtype=np.float32)[None, :], (P, C))),
    }
    in_maps = []
    for c in range(N_CORES):
        fs = features[B * c: B * (c + 1)]            # [B, T]
        ls = labels[B * c: B * (c + 1)]              # [B]
        ft = fs.T                                    # [T, B]
        featrep = np.ascontiguousarray(
            np.broadcast_to(ft[:, None, :], (T, C, B))).astype(wdt)
        labT = np.ascontiguousarray(ls.reshape(B // P, P).T.astype(np.float32))  # [P, B//P]
        in_maps.append({**shared, "featrep": featrep, "labelsT": labT})
    return in_maps


def gather_output(results):
    outs = []
    for r in results:
        outs.append(np.asarray(r["loss"]).T.reshape(-1))   # [P, B//P] -> [B]
    return np.concatenate(outs, axis=0).astype(np.float32)


def kernel(features, labels, embedding, W1, b1, W2, b2, Wd, bd):
    nc = _get_program()
    in_maps = make_in_maps(features, labels, embedding, W1, b1, W2, b2, Wd, bd)
    res = run_bass_kernel_spmd(nc, in_maps, core_ids=list(range(N_CORES)))
    return gather_output(res.results)



# revision 16
# speedup vs baseline: 1.1957x; 1.1957x over previous
"""Trainium2 Bass kernel: 2-layer LSTM (T=80, H=256) + embedding + softmax CE loss.

Strategy: data-parallel over batch (8192 -> 8 cores x 1024).  Everything runs
in a transposed layout: states/gates keep the hidden/gate dim on SBUF
partitions and the batch dim on the free axis, so the recurrent matmuls need
no per-step transposes (stationary = weights, moving = state).

The embedding lookup is reformulated as a one-hot matmul:
    x_t @ W1x  ==  onehot(feat_t) @ (emb @ W1x) = onehot @ E1
E1' = emb @ W1x + b1 + forget_bias_one_hot_fold, so layer-1 needs no bias adds
at all (each sample picks exactly one one-hot row).  The one-hot itself is an
is_equal compare against an iota column, with the feature row replicated
across 80 partitions host-side (pure layout prep).

A logical [256, 1024] tensor is stored "folded" as one SBUF tile [128, 2048]:
hidden unit u lives at (partition u % 128, col-block u // 128).  Gate g's
pre-activations accumulate in one PSUM tile [128, 2048] (4 banks): block ml
holds gate dims 256 g + 128 ml + p.

Final loss: logits computed as Wd.T @ h2 -> [80, B], PE-transposed back to
[B-chunk, 80] so log-sum-exp and the label gather run along the free axis.
"""

import sys

sys.path.insert(0, "/opt/trn_rl_repo")

import numpy as np

import concourse.bass as bass
import concourse.mybir as mybir
import concourse.tile as tile
from concourse import bacc
from concourse.bass_utils import run_bass_kernel_spmd

AF = mybir.ActivationFunctionType
OP = mybir.AluOpType
F32 = mybir.dt.float32
BF16 = mybir.dt.bfloat16
I32 = mybir.dt.int32
FP8 = mybir.dt.float8e4
DT = BF16          # dtype for weights / states / gate activations (matmul operands)
WS = 8.0           # fp8 weight pre-scale (undone by activation scale=1/WS)

P = 128          # partitions
N_CORES = 8
B = 1024         # per-core batch shard
T = 80           # seq len
C = 80           # num classes
E = 8            # emb dim
H = 256          # hidden
G = 4 * H        # gates = 1024
NB = B // 512    # moving-operand chunks of 512 (fp32 max free dim)

GATE_FUNCS = [AF.Sigmoid, AF.Tanh, AF.Sigmoid, AF.Sigmoid]  # i, j, f, o


def _blob16_offsets(FT):
    sizes = [("featrep", FT * C * B), ("emb", C * E), ("W1", (E + H) * G),
             ("b1", G), ("W2", 2 * H * G), ("Wd", H * C)]
    out, o = {}, 0
    for k, ln in sizes:
        out[k] = (o, ln)
        o += ln
    out["end"] = o
    return out


def _blob32_offsets():
    sizes = [("labelsT", P * (B // P)), ("b2", G), ("bd", C),
             ("ident80", C * C), ("iota80", C), ("iotalab", P * C)]
    out, o = {}, 0
    for k, ln in sizes:
        out[k] = (o, ln)
        o += ln
    out["end"] = o
    return out


def build_program(T_steps=T, thin=None, featrep_steps=None):
    # Bacc (not plain Bass): its compile() runs generate_event_semaphores,
    # which splits excess per-instruction sync waits onto InstEventSemaphore
    # nops — walrus only allows one wait on LDWEIGHTS/MATMULT.
    nc = bacc.Bacc("TRN2", target_bir_lowering=False, debug=False,
                   enable_asserts=False, num_devices=N_CORES)

    # ---------------- DRAM I/O ----------------
    # All inputs are packed into TWO flat buffers (one bf16, one f32): the
    # per-call PJRT/axon dispatch overhead is ~64us PER BUFFER, so 13 separate
    # inputs cost ~0.8ms/call in pure launch overhead.
    FT = featrep_steps or T_steps
    off16 = _blob16_offsets(FT)
    off32 = _blob32_offsets()
    blob16 = nc.dram_tensor("blob16", [off16["end"]], DT, kind="ExternalInput").ap()
    blob32 = nc.dram_tensor("blob32", [off32["end"]], F32, kind="ExternalInput").ap()

    def v16(key, shape_str=None, **axes):
        o, ln = off16[key]
        v = blob16[o:o + ln]
        return v.rearrange(shape_str, **axes) if shape_str else v

    def v32(key, shape_str=None, **axes):
        o, ln = off32[key]
        v = blob32[o:o + ln]
        return v.rearrange(shape_str, **axes) if shape_str else v

    featrep = v16("featrep", "(t c b) -> t c b", c=C, b=B)
    emb = v16("emb", "(c e) -> c e", e=E)
    W1d = v16("W1", "(r g) -> r g", g=G)
    b1d = v16("b1")
    W2d = v16("W2", "(r g) -> r g", g=G)
    Wdd = v16("Wd", "(r c) -> r c", c=C)
    labelsT = v32("labelsT", "(p m) -> p m", m=B // P)
    b2d = v32("b2")
    bdd = v32("bd")
    ident = v32("ident80", "(a b) -> a b", b=C)
    iota80d = v32("iota80", "(a b) -> a b", b=1)
    iotalabd = v32("iotalab", "(p c) -> p c", c=C)
    lossd = nc.dram_tensor("loss", [P, B // P], F32, kind="ExternalOutput").ap()

    with tile.TileContext(nc) as tc:
        _emit(nc, tc, featrep, labelsT, emb, W1d, b1d, W2d, b2d, Wdd, bdd,
              ident, iota80d, iotalabd, lossd, T_steps, thin)
    nc.compile()
    return nc


def _emit(nc, tc, featrep, labelsT, emb, W1d, b1d, W2d, b2d, Wdd, bdd,
          ident, iota80d, iotalabd, lossd, T_steps=T, thin=None):
    f32, i32 = F32, I32

    def act(out, in_, func, **kw):
        if thin == "act":
            nc.scalar.activation(out[:, 0:32], in_[:, 0:32], func, **kw)
        else:
            nc.scalar.activation(out, in_, func, **kw)

    def tt(out, a, b_, op):
        if thin == "dve":
            nc.vector.tensor_tensor(out[:, 0:32], a[:, 0:32], b_[:, 0:32], op=op)
        else:
            nc.vector.tensor_tensor(out, a, b_, op=op)

    const = tc.alloc_tile_pool(name="const", bufs=1)

    # ---------------- resident weights/constants ----------------
    # Recurrent weights are stored fp8e4 scaled by WS; the gate activations
    # apply scale=1/WS to undo it.  fp8 + DoubleRow runs the K=256 contraction
    # in ONE matmul at 0.5 cycles/row (2x PE throughput, half the instructions).
    W1h = const.tile([P, 2 * G], DT)          # k-chunk k at cols [G k, G k + G)
    for k in range(2):
        nc.sync.dma_start(out=W1h[:, k * G:(k + 1) * G],
                          in_=W1d[E + P * k: E + P * (k + 1), :])
    W2 = const.tile([P, 4 * G], DT)
    for k in range(4):
        nc.sync.dma_start(out=W2[:, k * G:(k + 1) * G],
                          in_=W2d[P * k: P * (k + 1), :])
    Wd = const.tile([P, 2 * C], DT)
    for k in range(2):
        nc.sync.dma_start(out=Wd[:, k * C:(k + 1) * C],
                          in_=Wdd[P * k: P * (k + 1), :])
    W1h8 = const.tile([P, 2 * G], FP8)
    nc.scalar.activation(W1h8, W1h, AF.Identity, scale=WS)
    W28 = const.tile([P, 4 * G], FP8)
    nc.scalar.activation(W28, W2, AF.Identity, scale=WS)
    Wd8 = const.tile([P, 2 * C], FP8)
    nc.scalar.activation(Wd8, Wd, AF.Identity, scale=WS)
    W1h8k = W1h8.rearrange("p (k g) -> p k g", k=2)
    W28k = W28.rearrange("p (k g) -> p k g", k=4)
    Wd8k = Wd8.rearrange("p (k c) -> p k c", k=2)
    b2c = const.tile([P, G // P], f32)          # col m = b2[128 m : 128 m + 128]
    nc.sync.dma_start(out=b2c, in_=b2d.rearrange("(m p) -> p m", p=P))
    b2rowf = const.tile([1, G], f32)
    nc.sync.dma_start(out=b2rowf, in_=b2d[None, :])
    b2row8 = const.tile([1, G], DT)             # WS * (b2 + forget one-hot)
    nc.vector.tensor_scalar(b2row8, b2rowf, WS, None, op0=OP.mult)
    nc.vector.tensor_scalar_add(b2row8[:, 512:768], b2row8[:, 512:768], WS)
    ones1b = const.tile([1, 512], DT)
    nc.vector.memset(ones1b, 1.0)
    bdc = const.tile([C, 1], f32)
    nc.sync.dma_start(out=bdc, in_=bdd[:, None])
    id80 = const.tile([C, C], f32)
    nc.sync.dma_start(out=id80, in_=ident)
    iota80 = const.tile([C, 1], f32)
    nc.sync.dma_start(out=iota80, in_=iota80d)
    iotalab = const.tile([P, C], f32)
    nc.sync.dma_start(out=iotalab, in_=iotalabd)
    labT = const.tile([P, B // P], f32)
    nc.sync.dma_start(out=labT, in_=labelsT)

    # forget-gate bias for layer 2: +1.0 on gate dims [512, 768) = cols 4,5
    nc.vector.tensor_scalar_add(b2c[:, 4:6], b2c[:, 4:6], 1.0)

    # ---------------- E1' = emb @ W1x + b1 (+1 on f-range) ----------------
    startup = tc.alloc_tile_pool(name="startup", bufs=1)
    embT = startup.tile([E, C], DT)
    nc.sync.dma_start(out=embT, in_=emb.rearrange("c e -> e c"))
    W1x = startup.tile([E, G], DT)
    nc.sync.dma_start(out=W1x, in_=W1d[0:E, :])
    b1row = startup.tile([1, G], DT)
    nc.sync.dma_start(out=b1row, in_=b1d[None, :])
    ones1 = startup.tile([1, C], DT)
    nc.vector.memset(ones1, 1.0)

    pstart = tc.alloc_tile_pool(name="pstart", bufs=1, space="PSUM")
    e1ps = pstart.tile([C, G], f32)
    for n in range(2):
        s = slice(512 * n, 512 * (n + 1))
        nc.tensor.matmul(e1ps[:, s], embT, W1x[:, s], start=True, stop=False)
        nc.tensor.matmul(e1ps[:, s], ones1, b1row[:, s], start=False, stop=True)
    E1 = const.tile([C, G], DT)
    nc.scalar.activation(E1, e1ps, AF.Identity, scale=WS)  # pre-scaled like W1h8
    nc.vector.tensor_scalar_add(E1[:, 512:768], E1[:, 512:768], WS)  # forget bias
    pstart.release()
    startup.release()

    # ---------------- pools for the recurrent loop ----------------
    states = tc.alloc_tile_pool(name="states", bufs=2)
    gates = tc.alloc_tile_pool(name="gates", bufs=2)
    pgate = tc.alloc_tile_pool(name="pgate", bufs=2, space="PSUM")
    feats = tc.alloc_tile_pool(name="feats", bufs=3)

    h1 = c1 = h2 = c2 = None
    SI = 1.0 / WS

    def l1_block(t, oh, h1_in):
        # One PSUM tile + ONE activation per gate ([P, 2B] = 4 banks).
        # Emitted in gate PAIRS with all (dependency-free) E1 matmuls first,
        # so the in-order PE queue can pre-run them while h1 is still being
        # produced, instead of stalling at the first DoubleRow matmul.
        h1k = h1_in.rearrange("p (k b) -> p k b", k=2) if t > 0 else None
        e1_only = (t == 0 or thin == "pe")
        sg1 = []
        for gp in range(2):
            pair = (2 * gp, 2 * gp + 1)
            ps_t = {}
            for g in pair:
                ps_t[g] = pgate.tile([P, 2 * B], f32, tag="g", name=f"ps1_{t}_{g}")
                for ml in range(2):
                    m = 2 * g + ml
                    for n in range(NB):
                        dst = ps_t[g][:, B * ml + 512 * n: B * ml + 512 * (n + 1)]
                        rhs_oh = oh[:, 512 * n: 512 * (n + 1)]
                        nc.tensor.matmul(dst, E1[:, P * m: P * (m + 1)], rhs_oh,
                                         start=True, stop=e1_only)
            for g in pair:
                if not e1_only:
                    for ml in range(2):
                        m = 2 * g + ml
                        for n in range(NB):
                            dst = ps_t[g][:, B * ml + 512 * n: B * ml + 512 * (n + 1)]
                            nc.tensor.matmul(
                                dst,
                                W1h8k[:, :, P * m: P * (m + 1)],
                                h1k[:, :, 512 * n: 512 * (n + 1)],
                                start=False, stop=True,
                                perf_mode=mybir.MatmulPerfMode.DoubleRow)
                sg = gates.tile([P, 2 * B], DT, tag=f"sg_{g}", name=f"sg1_{g}")
                act(sg, ps_t[g], GATE_FUNCS[g], scale=SI)
                sg1.append(sg)
        return sg1

    def l2_block(t, h1_in, h2_in):
        # Bias enters the PSUM group as a K=1 matmul (b2row8.T @ ones) so the
        # whole [P, 2B] gate tile takes ONE activation; the bias matmuls are
        # dependency-free, giving the in-order PE queue work to pre-run.
        h1k = h1_in.rearrange("p (k b) -> p k b", k=2)
        h2k = h2_in.rearrange("p (k b) -> p k b", k=2) if t > 0 else None
        kks = [0] if (t == 0 or thin == "pe") else [0, 1]
        sg2 = []
        for g in range(4):
            sg = gates.tile([P, 2 * B], DT, tag=f"sg_{g}", name=f"sg2_{g}")
            ps = pgate.tile([P, 2 * B], f32, tag="g", name=f"ps2_{t}_{g}")
            for ml in range(2):
                m = 2 * g + ml
                for n in range(NB):
                    dst = ps[:, B * ml + 512 * n: B * ml + 512 * (n + 1)]
                    nc.tensor.matmul(dst, b2row8[:, P * m: P * (m + 1)],
                                     ones1b, start=True, stop=False)
            for ml in range(2):
                m = 2 * g + ml
                for n in range(NB):
                    dst = ps[:, B * ml + 512 * n: B * ml + 512 * (n + 1)]
                    for ki, kk in enumerate(kks):
                        hsrck = h1k if kk == 0 else h2k
                        nc.tensor.matmul(
                            dst,
                            W28k[:, 2 * kk: 2 * kk + 2, P * m: P * (m + 1)],
                            hsrck[:, :, 512 * n: 512 * (n + 1)],
                            start=False, stop=(ki == len(kks) - 1),
                            perf_mode=mybir.MatmulPerfMode.DoubleRow)
            act(sg, ps, GATE_FUNCS[g], scale=SI)
            sg2.append(sg)
        return sg2

    def cell(t, sgates, c_in, ctag, htag, thname):
        si, sj, sf, so = sgates
        tt(si, si, sj, OP.mult)                  # si <- sigmoid(i)*tanh(j)
        cn = states.tile([P, 2 * B], DT, tag=ctag, name=ctag)
        if t == 0:
            nc.vector.tensor_copy(cn, si)
        else:
            tt(sf, c_in, sf, OP.mult)            # sf <- c * sigmoid(f+1)
            tt(cn, sf, si, OP.add)
        th = gates.tile([P, 2 * B], DT, tag="th", name=thname, bufs=2)
        act(th, cn, AF.Tanh)
        hn = states.tile([P, 2 * B], FP8, tag=htag, name=htag)
        tt(hn, th, so, OP.mult)
        return cn, hn

    # Software pipeline: L2 runs one step behind L1.  The L1 cell (which
    # produces h1(t), the recurrent critical path) is emitted BEFORE the L2
    # block so its tanh isn't queued behind 8 unrelated L2 activations on
    # the in-order ACT engine.
    sg2_pend = None
    n_feat = featrep.shape[0]
    for t in range(T_steps):
        featbc = feats.tile([C, B], DT, tag="featbc")
        nc.sync.dma_start(out=featbc, in_=featrep[t % n_feat])
        oh = feats.tile([C, B], DT, tag="oh")
        nc.vector.tensor_scalar(oh, featbc, iota80[:, 0:1], None, op0=OP.is_equal)

        h1_prev = h1
        sg1 = l1_block(t, oh, h1_prev)
        c1, h1 = cell(t, sg1, c1, "c1", "h1", "th1")
        if t > 0:
            sg2_pend = l2_block(t - 1, h1_prev, h2)
            c2, h2 = cell(t - 1, sg2_pend, c2, "c2", "h2", "th2")

    # drain the pipeline: L2 for the final step
    sg2_pend = l2_block(T_steps - 1, h1, h2)
    c2, h2 = cell(T_steps - 1, sg2_pend, c2, "c2", "h2", "th2")

    feats.release()
    pgate.release()

    # ---------------- loss ----------------
    ploss = tc.alloc_tile_pool(name="ploss", bufs=1, space="PSUM")
    lpool = tc.alloc_tile_pool(name="lpool", bufs=2)

    lps = ploss.tile([C, B], f32, tag="logits")
    h2k = h2.rearrange("p (k b) -> p k b", k=2)
    for n in range(NB):
        nc.tensor.matmul(
            lps[:, 512 * n: 512 * (n + 1)],
            Wd8k,
            h2k[:, :, 512 * n: 512 * (n + 1)],
            start=True, stop=True,
            perf_mode=mybir.MatmulPerfMode.DoubleRow)
    logits = lpool.tile([C, B], f32, tag="logits_sb", bufs=1)
    nc.scalar.activation(logits, lps, AF.Identity, bias=bdc[:, 0:1], scale=SI)

    loss_sb = lpool.tile([P, B // P], f32, tag="loss_sb", bufs=1)
    # per-chunk sum-exps / label logits gathered as COLUMNS of shared tiles so
    # the log and the final subtract are single ops (one Exp->Ln table switch)
    sumexp_all = lpool.tile([P, B // P], f32, tag="sumexp_all", bufs=1)
    lablog_all = lpool.tile([P, B // P], f32, tag="lablog_all", bufs=1)
    for cb in range(B // P):
        lt = ploss.tile([P, C], f32, tag="lt", bufs=2, name=f"lt_{cb}")
        nc.tensor.transpose(lt, logits[:, P * cb: P * (cb + 1)], id80)
        ohl = lpool.tile([P, C], f32, tag="ohl", name=f"ohl_{cb}")
        nc.vector.tensor_scalar(ohl, iotalab, labT[:, cb:cb + 1], None,
                                op0=OP.is_equal)
        scr1 = lpool.tile([P, C], f32, tag="scr1", name=f"scr1_{cb}")
        nc.vector.scalar_tensor_tensor(scr1, lt, 1.0, ohl,
                                       op0=OP.mult, op1=OP.mult,
                                       accum_out=lablog_all[:, cb:cb + 1])
        scr2 = lpool.tile([P, C], f32, tag="scr2", name=f"scr2_{cb}")
        nc.scalar.activation(scr2, lt, AF.Exp,
                             accum_out=sumexp_all[:, cb:cb + 1])
    lse = lpool.tile([P, B // P], f32, tag="lse", bufs=1)
    nc.scalar.activation(lse, sumexp_all, AF.Ln)
    nc.vector.tensor_sub(loss_sb, lse, lablog_all)
    nc.sync.dma_start(out=lossd, in_=loss_sb)
    lpool.release()
    ploss.release()
    gates.release()
    states.release()
    const.release()


# ---------------------------------------------------------------------------
# host side
# ---------------------------------------------------------------------------
_CACHE = {}


def _get_program():
    if "nc" not in _CACHE:
        _CACHE["nc"] = build_program()
    return _CACHE["nc"]


def make_in_maps(features, labels, embedding, W1, b1, W2, b2, Wd, bd,
                 T_steps=T):
    """Shard the full inputs into 8 per-core input maps (2 packed blobs)."""
    features = np.asarray(features, dtype=np.int32)
    labels = np.asarray(labels, dtype=np.int32)
    import ml_dtypes
    wdt = ml_dtypes.bfloat16 if DT == BF16 else np.float32
    shared16 = [
        np.asarray(embedding, np.float32).astype(wdt).ravel(),
        np.asarray(W1, np.float32).astype(wdt).ravel(),
        np.asarray(b1, np.float32).astype(wdt).ravel(),
        np.asarray(W2, np.float32).astype(wdt).ravel(),
        np.asarray(Wd, np.float32).astype(wdt).ravel(),
    ]
    shared32_tail = [
        np.asarray(b2, np.float32).ravel(),
        np.asarray(bd, np.float32).ravel(),
        np.eye(C, dtype=np.float32).ravel(),
        np.arange(C, dtype=np.float32).ravel(),
        np.ascontiguousarray(
            np.broadcast_to(np.arange(C, dtype=np.float32)[None, :], (P, C))).ravel(),
    ]
    in_maps = []
    for c in range(N_CORES):
        fs = features[B * c: B * (c + 1)]            # [B, T]
        ls = labels[B * c: B * (c + 1)]              # [B]
        ft = fs.T[:T_steps]                          # [T_steps, B]
        featrep = np.ascontiguousarray(
            np.broadcast_to(ft[:, None, :], (T_steps, C, B))).astype(wdt)
        labT = np.ascontiguousarray(ls.reshape(B // P, P).T.astype(np.float32))
        blob16 = np.concatenate([featrep.ravel()] + shared16)
        blob32 = np.concatenate([labT.ravel()] + shared32_tail)
        in_maps.append({"blob16": blob16, "blob32": blob32})
    return in_maps


def gather_output(results):
    outs = []
    for r in results:
        outs.append(np.asarray(r["loss"]).T.reshape(-1))   # [P, B//P] -> [B]
    return np.concatenate(outs, axis=0).astype(np.float32)


def kernel(features, labels, embedding, W1, b1, W2, b2, Wd, bd):
    nc = _get_program()
    in_maps = make_in_maps(features, labels, embedding, W1, b1, W2, b2, Wd, bd)
    res = run_bass_kernel_spmd(nc, in_maps, core_ids=list(range(N_CORES)))
    return gather_output(res.results)



# revision 20
# speedup vs baseline: 1.2911x; 1.0798x over previous
"""Trainium2 Bass kernel: 2-layer LSTM (T=80, H=256) + embedding + softmax CE loss.

Strategy: data-parallel over batch (8192 -> 8 cores x 1024).  Everything runs
in a transposed layout: states/gates keep the hidden/gate dim on SBUF
partitions and the batch dim on the free axis, so the recurrent matmuls need
no per-step transposes (stationary = weights, moving = state).

The embedding lookup is reformulated as a one-hot matmul:
    x_t @ W1x  ==  onehot(feat_t) @ (emb @ W1x) = onehot @ E1
E1' = WS*(emb @ W1x + b1 + forget_bias_one_hot_fold), so layer-1 needs no
bias adds at all.  The one-hot itself is an is_equal compare against an iota
column, with the feature row replicated across 80 partitions host-side.

A logical [256, 1024] tensor is stored "folded" as one SBUF tile [128, 2048]:
hidden unit u lives at (partition u % 128, col-block u // 128).

Perf structure (HW-validated at ~24 us/step, ACT-engine-bound):
- Recurrent weights + h states are fp8e4 scaled by WS=8; the K=256 (L1) and
  K=512 (L2) contractions run as DoubleRow matmuls (2 fp8 weights/PE cell ->
  half the instructions, 0.5 cyc/row).  Gate activations apply scale=1/WS.
  The folded [128, 2, B] state layout is exactly DoubleRow's 3D-AP shape.
- Per L1 gate: ONE [128, 2B] PSUM tile (4 banks) + ONE activation.  L2 acts
  stay per-half because the act's bias operand is per-partition ([P,1]) and
  the two halves need different b2 slices.
- Iteration emits l1_block -> L1 cell -> l2_block -> L2 cell (L2 one step
  behind), so the tanh on the h1 critical path is not queued behind 8
  unrelated L2 activations on the in-order ACT engine; L1 emits its
  dependency-free E1 matmuls for a gate PAIR before the h1-dependent
  DoubleRow matmuls so the in-order PE queue pre-runs them.
- All 13 logical inputs are packed into 2 flat DRAM blobs (bf16 + f32):
  per-call dispatch overhead is ~64us PER INPUT BUFFER (~0.8ms for 13).

Final loss: logits via one DoubleRow matmul Wd.T @ h2 -> [80, B],
PE-transposed back to [B-chunk, 80] so log-sum-exp and the label gather run
along the free axis.
"""

import sys

sys.path.insert(0, "/opt/trn_rl_repo")

import numpy as np

import concourse.bass as bass
import concourse.mybir as mybir
import concourse.tile as tile
from concourse import bacc
from concourse.bass_utils import run_bass_kernel_spmd

AF = mybir.ActivationFunctionType
OP = mybir.AluOpType
F32 = mybir.dt.float32
BF16 = mybir.dt.bfloat16
I32 = mybir.dt.int32
FP8 = mybir.dt.float8e4
DT = BF16          # dtype for weights / states / gate activations (matmul operands)
WS = 8.0           # fp8 weight pre-scale (undone by activation scale=1/WS)

P = 128          # partitions
N_CORES = 8
B = 1024         # per-core batch shard
T = 80           # seq len
C = 80           # num classes
E = 8            # emb dim
H = 256          # hidden
G = 4 * H        # gates = 1024
NB = B // 512    # moving-operand chunks of 512 (fp32 max free dim)

GATE_FUNCS = [AF.Sigmoid, AF.Tanh, AF.Sigmoid, AF.Sigmoid]  # i, j, f, o


def _blob16_offsets(FT):
    sizes = [("featrep", FT * C * B), ("emb", C * E), ("W1", (E + H) * G),
             ("b1", G), ("W2", 2 * H * G), ("Wd", H * C)]
    out, o = {}, 0
    for k, ln in sizes:
        out[k] = (o, ln)
        o += ln
    out["end"] = o
    return out


def _blob32_offsets():
    sizes = [("labelsT", P * (B // P)), ("b2", G), ("bd", C),
             ("ident80", C * C), ("iota80", C), ("iotalab", P * C)]
    out, o = {}, 0
    for k, ln in sizes:
        out[k] = (o, ln)
        o += ln
    out["end"] = o
    return out


def build_program(T_steps=T, thin=None, featrep_steps=None):
    # Bacc (not plain Bass): its compile() runs generate_event_semaphores,
    # which splits excess per-instruction sync waits onto InstEventSemaphore
    # nops — walrus only allows one wait on LDWEIGHTS/MATMULT.
    nc = bacc.Bacc("TRN2", target_bir_lowering=False, debug=False,
                   enable_asserts=False, num_devices=N_CORES)

    # ---------------- DRAM I/O ----------------
    # All inputs are packed into TWO flat buffers (one bf16, one f32): the
    # per-call PJRT/axon dispatch overhead is ~64us PER BUFFER, so 13 separate
    # inputs cost ~0.8ms/call in pure launch overhead.
    FT = featrep_steps or T_steps
    off16 = _blob16_offsets(FT)
    off32 = _blob32_offsets()
    blob16 = nc.dram_tensor("blob16", [off16["end"]], DT, kind="ExternalInput").ap()
    blob32 = nc.dram_tensor("blob32", [off32["end"]], F32, kind="ExternalInput").ap()

    def v16(key, shape_str=None, **axes):
        o, ln = off16[key]
        v = blob16[o:o + ln]
        return v.rearrange(shape_str, **axes) if shape_str else v

    def v32(key, shape_str=None, **axes):
        o, ln = off32[key]
        v = blob32[o:o + ln]
        return v.rearrange(shape_str, **axes) if shape_str else v

    featrep = v16("featrep", "(t c b) -> t c b", c=C, b=B)
    emb = v16("emb", "(c e) -> c e", e=E)
    W1d = v16("W1", "(r g) -> r g", g=G)
    b1d = v16("b1")
    W2d = v16("W2", "(r g) -> r g", g=G)
    Wdd = v16("Wd", "(r c) -> r c", c=C)
    labelsT = v32("labelsT", "(p m) -> p m", m=B // P)
    b2d = v32("b2")
    bdd = v32("bd")
    ident = v32("ident80", "(a b) -> a b", b=C)
    iota80d = v32("iota80", "(a b) -> a b", b=1)
    iotalabd = v32("iotalab", "(p c) -> p c", c=C)
    lossd = nc.dram_tensor("loss", [P, B // P], F32, kind="ExternalOutput").ap()

    with tile.TileContext(nc) as tc:
        _emit(nc, tc, featrep, labelsT, emb, W1d, b1d, W2d, b2d, Wdd, bdd,
              ident, iota80d, iotalabd, lossd, T_steps, thin)
    nc.compile()
    return nc


def _emit(nc, tc, featrep, labelsT, emb, W1d, b1d, W2d, b2d, Wdd, bdd,
          ident, iota80d, iotalabd, lossd, T_steps=T, thin=None):
    f32, i32 = F32, I32

    def act(out, in_, func, **kw):
        if thin == "act":
            nc.scalar.activation(out[:, 0:32], in_[:, 0:32], func, **kw)
        else:
            nc.scalar.activation(out, in_, func, **kw)

    def tt(out, a, b_, op):
        if thin == "dve":
            nc.vector.tensor_tensor(out[:, 0:32], a[:, 0:32], b_[:, 0:32], op=op)
        else:
            nc.vector.tensor_tensor(out, a, b_, op=op)

    const = tc.alloc_tile_pool(name="const", bufs=1)

    # ---------------- resident weights/constants ----------------
    # Recurrent weights are stored fp8e4 scaled by WS; the gate activations
    # apply scale=1/WS to undo it.  fp8 + DoubleRow runs the K=256 contraction
    # in ONE matmul at 0.5 cycles/row (2x PE throughput, half the instructions).
    W1h = const.tile([P, 2 * G], DT)          # k-chunk k at cols [G k, G k + G)
    for k in range(2):
        nc.sync.dma_start(out=W1h[:, k * G:(k + 1) * G],
                          in_=W1d[E + P * k: E + P * (k + 1), :])
    W2 = const.tile([P, 4 * G], DT)
    for k in range(4):
        nc.sync.dma_start(out=W2[:, k * G:(k + 1) * G],
                          in_=W2d[P * k: P * (k + 1), :])
    Wd = const.tile([P, 2 * C], DT)
    for k in range(2):
        nc.sync.dma_start(out=Wd[:, k * C:(k + 1) * C],
                          in_=Wdd[P * k: P * (k + 1), :])
    W1h8 = const.tile([P, 2 * G], FP8)
    nc.scalar.activation(W1h8, W1h, AF.Identity, scale=WS)
    W28 = const.tile([P, 4 * G], FP8)
    nc.scalar.activation(W28, W2, AF.Identity, scale=WS)
    Wd8 = const.tile([P, 2 * C], FP8)
    nc.scalar.activation(Wd8, Wd, AF.Identity, scale=WS)
    W1h8k = W1h8.rearrange("p (k g) -> p k g", k=2)
    W28k = W28.rearrange("p (k g) -> p k g", k=4)
    Wd8k = Wd8.rearrange("p (k c) -> p k c", k=2)
    b2c = const.tile([P, G // P], f32)          # col m = b2[128 m : 128 m + 128]
    nc.sync.dma_start(out=b2c, in_=b2d.rearrange("(m p) -> p m", p=P))

    bdc = const.tile([C, 1], f32)
    nc.sync.dma_start(out=bdc, in_=bdd[:, None])
    id80 = const.tile([C, C], f32)
    nc.sync.dma_start(out=id80, in_=ident)
    iota80 = const.tile([C, 1], f32)
    nc.sync.dma_start(out=iota80, in_=iota80d)
    iotalab = const.tile([P, C], f32)
    nc.sync.dma_start(out=iotalab, in_=iotalabd)
    labT = const.tile([P, B // P], f32)
    nc.sync.dma_start(out=labT, in_=labelsT)

    # forget-gate bias for layer 2: +1.0 on gate dims [512, 768) = cols 4,5
    nc.vector.tensor_scalar_add(b2c[:, 4:6], b2c[:, 4:6], 1.0)

    # ---------------- E1' = emb @ W1x + b1 (+1 on f-range) ----------------
    startup = tc.alloc_tile_pool(name="startup", bufs=1)
    embT = startup.tile([E, C], DT)
    nc.sync.dma_start(out=embT, in_=emb.rearrange("c e -> e c"))
    W1x = startup.tile([E, G], DT)
    nc.sync.dma_start(out=W1x, in_=W1d[0:E, :])
    b1row = startup.tile([1, G], DT)
    nc.sync.dma_start(out=b1row, in_=b1d[None, :])
    ones1 = startup.tile([1, C], DT)
    nc.vector.memset(ones1, 1.0)

    pstart = tc.alloc_tile_pool(name="pstart", bufs=1, space="PSUM")
    e1ps = pstart.tile([C, G], f32)
    for n in range(2):
        s = slice(512 * n, 512 * (n + 1))
        nc.tensor.matmul(e1ps[:, s], embT, W1x[:, s], start=True, stop=False)
        nc.tensor.matmul(e1ps[:, s], ones1, b1row[:, s], start=False, stop=True)
    E1 = const.tile([C, G], DT)
    nc.scalar.activation(E1, e1ps, AF.Identity, scale=WS)  # pre-scaled like W1h8
    nc.vector.tensor_scalar_add(E1[:, 512:768], E1[:, 512:768], WS)  # forget bias
    pstart.release()
    startup.release()

    # ---------------- pools for the recurrent loop ----------------
    states = tc.alloc_tile_pool(name="states", bufs=2)
    gates = tc.alloc_tile_pool(name="gates", bufs=2)
    pgate = tc.alloc_tile_pool(name="pgate", bufs=2, space="PSUM")
    feats = tc.alloc_tile_pool(name="feats", bufs=3)

    h1 = c1 = h2 = c2 = None
    SI = 1.0 / WS

    def l1_block(t, oh, h1_in):
        # One PSUM tile + ONE activation per gate ([P, 2B] = 4 banks).
        # Emitted in gate PAIRS with all (dependency-free) E1 matmuls first,
        # so the in-order PE queue can pre-run them while h1 is still being
        # produced, instead of stalling at the first DoubleRow matmul.
        h1k = h1_in.rearrange("p (k b) -> p k b", k=2) if t > 0 else None
        e1_only = (t == 0 or thin == "pe")
        sg1 = []
        for gp in range(2):
            pair = (2 * gp, 2 * gp + 1)
            ps_t = {}
            for g in pair:
                ps_t[g] = pgate.tile([P, 2 * B], f32, tag="g", name=f"ps1_{t}_{g}")
                for ml in range(2):
                    m = 2 * g + ml
                    for n in range(NB):
                        dst = ps_t[g][:, B * ml + 512 * n: B * ml + 512 * (n + 1)]
                        rhs_oh = oh[:, 512 * n: 512 * (n + 1)]
                        nc.tensor.matmul(dst, E1[:, P * m: P * (m + 1)], rhs_oh,
                                         start=True, stop=e1_only)
            for g in pair:
                if not e1_only:
                    for ml in range(2):
                        m = 2 * g + ml
                        for n in range(NB):
                            dst = ps_t[g][:, B * ml + 512 * n: B * ml + 512 * (n + 1)]
                            nc.tensor.matmul(
                                dst,
                                W1h8k[:, :, P * m: P * (m + 1)],
                                h1k[:, :, 512 * n: 512 * (n + 1)],
                                start=False, stop=True,
                                perf_mode=mybir.MatmulPerfMode.DoubleRow)
                sg = gates.tile([P, 2 * B], DT, tag=f"sg_{g}", name=f"sg1_{g}")
                act(sg, ps_t[g], GATE_FUNCS[g], scale=SI)
                sg1.append(sg)
        return sg1

    def l2_block(t, h1_in, h2_in):
        h1k = h1_in.rearrange("p (k b) -> p k b", k=2)
        h2k = h2_in.rearrange("p (k b) -> p k b", k=2) if t > 0 else None
        kks = [0] if (t == 0 or thin == "pe") else [0, 1]
        sg2 = []
        for g in range(4):
            sg = gates.tile([P, 2 * B], DT, tag=f"sg_{g}", name=f"sg2_{g}")
            ps = pgate.tile([P, 2 * B], f32, tag="g", name=f"ps2_{t}_{g}")
            for ml in range(2):
                m = 2 * g + ml
                # kk outer so consecutive matmuls share the stationary
                # operand (one LDWEIGHTS per weight tile, not per matmul)
                for ki, kk in enumerate(kks):
                    hsrck = h1k if kk == 0 else h2k
                    for n in range(NB):
                        dst = ps[:, B * ml + 512 * n: B * ml + 512 * (n + 1)]
                        nc.tensor.matmul(
                            dst,
                            W28k[:, 2 * kk: 2 * kk + 2, P * m: P * (m + 1)],
                            hsrck[:, :, 512 * n: 512 * (n + 1)],
                            start=(ki == 0), stop=(ki == len(kks) - 1),
                            perf_mode=mybir.MatmulPerfMode.DoubleRow)
                # per-m bias differs across the two halves -> act per half
                act(sg[:, ml * B:(ml + 1) * B], ps[:, ml * B:(ml + 1) * B],
                    GATE_FUNCS[g], bias=b2c[:, m:m + 1], scale=SI)
            sg2.append(sg)
        return sg2

    def cell(t, sgates, c_in, ctag, htag, thname):
        si, sj, sf, so = sgates
        tt(si, si, sj, OP.mult)                  # si <- sigmoid(i)*tanh(j)
        cn = states.tile([P, 2 * B], DT, tag=ctag, name=ctag)
        if t == 0:
            nc.vector.tensor_copy(cn, si)
        else:
            tt(sf, c_in, sf, OP.mult)            # sf <- c * sigmoid(f+1)
            tt(cn, sf, si, OP.add)
        th = gates.tile([P, 2 * B], DT, tag="th", name=thname, bufs=2)
        act(th, cn, AF.Tanh)
        hn = states.tile([P, 2 * B], FP8, tag=htag, name=htag)
        tt(hn, th, so, OP.mult)
        return cn, hn

    # Software pipeline: L2 runs one step behind L1.  The L1 cell (which
    # produces h1(t), the recurrent critical path) is emitted BEFORE the L2
    # block so its tanh isn't queued behind 8 unrelated L2 activations on
    # the in-order ACT engine.
    sg2_pend = None
    n_feat = featrep.shape[0]
    for t in range(T_steps):
        featbc = feats.tile([C, B], DT, tag="featbc")
        nc.sync.dma_start(out=featbc, in_=featrep[t % n_feat])
        oh = feats.tile([C, B], DT, tag="oh")
        nc.vector.tensor_scalar(oh, featbc, iota80[:, 0:1], None, op0=OP.is_equal)

        h1_prev = h1
        sg1 = l1_block(t, oh, h1_prev)
        c1, h1 = cell(t, sg1, c1, "c1", "h1", "th1")
        if t > 0:
            sg2_pend = l2_block(t - 1, h1_prev, h2)
            c2, h2 = cell(t - 1, sg2_pend, c2, "c2", "h2", "th2")

    # drain the pipeline: L2 for the final step
    sg2_pend = l2_block(T_steps - 1, h1, h2)
    c2, h2 = cell(T_steps - 1, sg2_pend, c2, "c2", "h2", "th2")

    feats.release()
    pgate.release()

    # ---------------- loss ----------------
    ploss = tc.alloc_tile_pool(name="ploss", bufs=1, space="PSUM")
    lpool = tc.alloc_tile_pool(name="lpool", bufs=2)

    lps = ploss.tile([C, B], f32, tag="logits")
    h2k = h2.rearrange("p (k b) -> p k b", k=2)
    for n in range(NB):
        nc.tensor.matmul(
            lps[:, 512 * n: 512 * (n + 1)],
            Wd8k,
            h2k[:, :, 512 * n: 512 * (n + 1)],
            start=True, stop=True,
            perf_mode=mybir.MatmulPerfMode.DoubleRow)
    logits = lpool.tile([C, B], f32, tag="logits_sb", bufs=1)
    nc.scalar.activation(logits, lps, AF.Identity, bias=bdc[:, 0:1], scale=SI)

    loss_sb = lpool.tile([P, B // P], f32, tag="loss_sb", bufs=1)
    # per-chunk sum-exps / label logits gathered as COLUMNS of shared tiles so
    # the log and the final subtract are single ops (one Exp->Ln table switch)
    sumexp_all = lpool.tile([P, B // P], f32, tag="sumexp_all", bufs=1)
    lablog_all = lpool.tile([P, B // P], f32, tag="lablog_all", bufs=1)
    for cb in range(B // P):
        lt = ploss.tile([P, C], f32, tag="lt", bufs=2, name=f"lt_{cb}")
        nc.tensor.transpose(lt, logits[:, P * cb: P * (cb + 1)], id80)
        ohl = lpool.tile([P, C], f32, tag="ohl", name=f"ohl_{cb}")
        nc.vector.tensor_scalar(ohl, iotalab, labT[:, cb:cb + 1], None,
                                op0=OP.is_equal)
        scr1 = lpool.tile([P, C], f32, tag="scr1", name=f"scr1_{cb}")
        nc.vector.scalar_tensor_tensor(scr1, lt, 1.0, ohl,
                                       op0=OP.mult, op1=OP.mult,
                                       accum_out=lablog_all[:, cb:cb + 1])
        scr2 = lpool.tile([P, C], f32, tag="scr2", name=f"scr2_{cb}")
        nc.scalar.activation(scr2, lt, AF.Exp,
                             accum_out=sumexp_all[:, cb:cb + 1])
    lse = lpool.tile([P, B // P], f32, tag="lse", bufs=1)
    nc.scalar.activation(lse, sumexp_all, AF.Ln)
    nc.vector.tensor_sub(loss_sb, lse, lablog_all)
    nc.sync.dma_start(out=lossd, in_=loss_sb)
    lpool.release()
    ploss.release()
    gates.release()
    states.release()
    const.release()


# ---------------------------------------------------------------------------
# host side
# ---------------------------------------------------------------------------
_CACHE = {}


def _get_program():
    if "nc" not in _CACHE:
        _CACHE["nc"] = build_program()
    return _CACHE["nc"]


def make_in_maps(features, labels, embedding, W1, b1, W2, b2, Wd, bd,
                 T_steps=T):
    """Shard the full inputs into 8 per-core input maps (2 packed blobs)."""
    features = np.asarray(features, dtype=np.int32)
    labels = np.asarray(labels, dtype=np.int32)
    import ml_dtypes
    wdt = ml_dtypes.bfloat16 if DT == BF16 else np.float32
    shared16 = [
        np.asarray(embedding, np.float32).astype(wdt).ravel(),
        np.asarray(W1, np.float32).astype(wdt).ravel(),
        np.asarray(b1, np.float32).astype(wdt).ravel(),
        np.asarray(W2, np.float32).astype(wdt).ravel(),
        np.asarray(Wd, np.float32).astype(wdt).ravel(),
    ]
    shared32_tail = [
        np.asarray(b2, np.float32).ravel(),
        np.asarray(bd, np.float32).ravel(),
        np.eye(C, dtype=np.float32).ravel(),
        np.arange(C, dtype=np.float32).ravel(),
        np.ascontiguousarray(
            np.broadcast_to(np.arange(C, dtype=np.float32)[None, :], (P, C))).ravel(),
    ]
    in_maps = []
    for c in range(N_CORES):
        fs = features[B * c: B * (c + 1)]            # [B, T]
        ls = labels[B * c: B * (c + 1)]              # [B]
        ft = fs.T[:T_steps]                          # [T_steps, B]
        featrep = np.ascontiguousarray(
            np.broadcast_to(ft[:, None, :], (T_steps, C, B))).astype(wdt)
        labT = np.ascontiguousarray(ls.reshape(B // P, P).T.astype(np.float32))
        blob16 = np.concatenate([featrep.ravel()] + shared16)
        blob32 = np.concatenate([labT.ravel()] + shared32_tail)
        in_maps.append({"blob16": blob16, "blob32": blob32})
    return in_maps


def gather_output(results):
    outs = []
    for r in results:
        outs.append(np.asarray(r["loss"]).T.reshape(-1))   # [P, B//P] -> [B]
    return np.concatenate(outs, axis=0).astype(np.float32)


def kernel(features, labels, embedding, W1, b1, W2, b2, Wd, bd):
    nc = _get_program()
    in_maps = make_in_maps(features, labels, embedding, W1, b1, W2, b2, Wd, bd)
    res = run_bass_kernel_spmd(nc, in_maps, core_ids=list(range(N_CORES)))
    return gather_output(res.results)



# revision 35
# speedup vs baseline: 1.4272x; 1.1054x over previous
"""Trainium2 Bass kernel: 2-layer LSTM (T=80, H=256) + embedding + softmax CE loss.

Strategy: data-parallel over batch (8192 -> 8 cores x 1024).  Everything runs
in a transposed layout: states/gates keep the hidden/gate dim on SBUF
partitions and the batch dim on the free axis, so the recurrent matmuls need
no per-step transposes (stationary = weights, moving = state).

The embedding lookup is reformulated as a one-hot matmul:
    x_t @ W1x  ==  onehot(feat_t) @ (emb @ W1x) = onehot @ E1
E1' = WS*(emb @ W1x + b1 + forget_bias_one_hot_fold), so layer-1 needs no
bias adds at all.  The one-hot itself is an is_equal compare against an iota
column, with the feature row replicated across 80 partitions host-side.

A logical [256, 1024] tensor is stored "folded" as one SBUF tile [128, 2048]:
hidden unit u lives at (partition u % 128, col-block u // 128).

Perf structure (HW-validated at ~24 us/step, ACT-engine-bound):
- Recurrent weights + h states are fp8e4 scaled by WS=8; the K=256 (L1) and
  K=512 (L2) contractions run as DoubleRow matmuls (2 fp8 weights/PE cell ->
  half the instructions, 0.5 cyc/row).  Gate activations apply scale=1/WS.
  The folded [128, 2, B] state layout is exactly DoubleRow's 3D-AP shape.
- Per L1 gate: ONE [128, 2B] PSUM tile (4 banks) + ONE activation.  L2 acts
  stay per-half because the act's bias operand is per-partition ([P,1]) and
  the two halves need different b2 slices.
- Iteration emits l1_block -> L1 cell -> l2_block -> L2 cell (L2 one step
  behind), so the tanh on the h1 critical path is not queued behind 8
  unrelated L2 activations on the in-order ACT engine; L1 emits its
  dependency-free E1 matmuls for a gate PAIR before the h1-dependent
  DoubleRow matmuls so the in-order PE queue pre-runs them.
- All 13 logical inputs are packed into 2 flat DRAM blobs (bf16 + f32):
  per-call dispatch overhead is ~64us PER INPUT BUFFER (~0.8ms for 13).

Final loss: logits via one DoubleRow matmul Wd.T @ h2 -> [80, B],
PE-transposed back to [B-chunk, 80] so log-sum-exp and the label gather run
along the free axis.
"""

import sys

sys.path.insert(0, "/opt/trn_rl_repo")

import numpy as np

import concourse.bass as bass
import concourse.mybir as mybir
import concourse.tile as tile
from concourse import bacc
from concourse.bass_utils import run_bass_kernel_spmd

AF = mybir.ActivationFunctionType
OP = mybir.AluOpType
F32 = mybir.dt.float32
BF16 = mybir.dt.bfloat16
I32 = mybir.dt.int32
FP8 = mybir.dt.float8e4
DT = BF16          # dtype for weights / states / gate activations (matmul operands)
WS = 8.0           # fp8 weight pre-scale (undone by activation scale=1/WS)

P = 128          # partitions
N_CORES = 8
B = 1024         # per-core batch shard
T = 80           # seq len
C = 80           # num classes
E = 8            # emb dim
H = 256          # hidden
G = 4 * H        # gates = 1024
NB = B // 512    # moving-operand chunks of 512 (fp32 max free dim)

GATE_FUNCS = [AF.Sigmoid, AF.Tanh, AF.Sigmoid, AF.Sigmoid]  # i, j, f, o


def _blob16_offsets(FT):
    sizes = [("featrep", FT * C * B), ("emb", C * E), ("W1", (E + H) * G),
             ("b1", G), ("W2", 2 * H * G), ("Wd", H * C)]
    out, o = {}, 0
    for k, ln in sizes:
        out[k] = (o, ln)
        o += ln
    out["end"] = o
    return out


def _blob32_offsets():
    sizes = [("labelsT", P * (B // P)), ("b2", G), ("bd", C),
             ("ident80", C * C), ("iota80", C), ("iotalab", P * C)]
    out, o = {}, 0
    for k, ln in sizes:
        out[k] = (o, ln)
        o += ln
    out["end"] = o
    return out


def build_program(T_steps=T, thin=None, featrep_steps=None, zero_b2=False):
    # Bacc (not plain Bass): its compile() runs generate_event_semaphores,
    # which splits excess per-instruction sync waits onto InstEventSemaphore
    # nops — walrus only allows one wait on LDWEIGHTS/MATMULT.
    nc = bacc.Bacc("TRN2", target_bir_lowering=False, debug=False,
                   enable_asserts=False, num_devices=N_CORES)

    # ---------------- DRAM I/O ----------------
    # All inputs are packed into TWO flat buffers (one bf16, one f32): the
    # per-call PJRT/axon dispatch overhead is ~64us PER BUFFER, so 13 separate
    # inputs cost ~0.8ms/call in pure launch overhead.
    FT = featrep_steps or T_steps
    off16 = _blob16_offsets(FT)
    off32 = _blob32_offsets()
    blob16 = nc.dram_tensor("blob16", [off16["end"]], DT, kind="ExternalInput").ap()
    blob32 = nc.dram_tensor("blob32", [off32["end"]], F32, kind="ExternalInput").ap()

    def v16(key, shape_str=None, **axes):
        o, ln = off16[key]
        v = blob16[o:o + ln]
        return v.rearrange(shape_str, **axes) if shape_str else v

    def v32(key, shape_str=None, **axes):
        o, ln = off32[key]
        v = blob32[o:o + ln]
        return v.rearrange(shape_str, **axes) if shape_str else v

    featrep = v16("featrep", "(t c b) -> t c b", c=C, b=B)
    emb = v16("emb", "(c e) -> c e", e=E)
    W1d = v16("W1", "(r g) -> r g", g=G)
    b1d = v16("b1")
    W2d = v16("W2", "(r g) -> r g", g=G)
    Wdd = v16("Wd", "(r c) -> r c", c=C)
    labelsT = v32("labelsT", "(p m) -> p m", m=B // P)
    b2d = v32("b2")
    bdd = v32("bd")
    ident = v32("ident80", "(a b) -> a b", b=C)
    iota80d = v32("iota80", "(a b) -> a b", b=1)
    iotalabd = v32("iotalab", "(p c) -> p c", c=C)
    lossd = nc.dram_tensor("loss", [P, B // P], F32, kind="ExternalOutput").ap()

    with tile.TileContext(nc) as tc:
        _emit(nc, tc, featrep, labelsT, emb, W1d, b1d, W2d, b2d, Wdd, bdd,
              ident, iota80d, iotalabd, lossd, T_steps, thin, zero_b2)
    nc.compile()
    return nc


def _emit(nc, tc, featrep, labelsT, emb, W1d, b1d, W2d, b2d, Wdd, bdd,
          ident, iota80d, iotalabd, lossd, T_steps=T, thin=None,
          zero_b2=False):
    f32, i32 = F32, I32

    def act(out, in_, func, **kw):
        if thin == "act":
            nc.scalar.activation(out[:, 0:32], in_[:, 0:32], func, **kw)
        else:
            nc.scalar.activation(out, in_, func, **kw)

    def tt(out, a, b_, op):
        if thin == "dve":
            nc.vector.tensor_tensor(out[:, 0:32], a[:, 0:32], b_[:, 0:32], op=op)
        else:
            nc.vector.tensor_tensor(out, a, b_, op=op)

    const = tc.alloc_tile_pool(name="const", bufs=1)

    # ---------------- resident weights/constants ----------------
    # Recurrent weights are stored fp8e4 scaled by WS; the gate activations
    # apply scale=1/WS to undo it.  fp8 + DoubleRow runs the K=256 contraction
    # in ONE matmul at 0.5 cycles/row (2x PE throughput, half the instructions).
    W1h = const.tile([P, 2 * G], DT)          # k-chunk k at cols [G k, G k + G)
    for k in range(2):
        nc.sync.dma_start(out=W1h[:, k * G:(k + 1) * G],
                          in_=W1d[E + P * k: E + P * (k + 1), :])
    W2 = const.tile([P, 4 * G], DT)
    for k in range(4):
        nc.sync.dma_start(out=W2[:, k * G:(k + 1) * G],
                          in_=W2d[P * k: P * (k + 1), :])
    Wd = const.tile([P, 2 * C], DT)
    for k in range(2):
        nc.sync.dma_start(out=Wd[:, k * C:(k + 1) * C],
                          in_=Wdd[P * k: P * (k + 1), :])
    W1h8 = const.tile([P, 2 * G], FP8)
    nc.scalar.activation(W1h8, W1h, AF.Identity, scale=WS)
    W28 = const.tile([P, 4 * G], FP8)
    nc.scalar.activation(W28, W2, AF.Identity, scale=WS)
    Wd8 = const.tile([P, 2 * C], FP8)
    nc.scalar.activation(Wd8, Wd, AF.Identity, scale=WS)
    W1h8k = W1h8.rearrange("p (k g) -> p k g", k=2)
    W28k = W28.rearrange("p (k g) -> p k g", k=4)
    Wd8k = Wd8.rearrange("p (k c) -> p k c", k=2)
    b2c = const.tile([P, G // P], f32)          # col m = b2[128 m : 128 m + 128]
    nc.sync.dma_start(out=b2c, in_=b2d.rearrange("(m p) -> p m", p=P))

    bdc = const.tile([C, 1], f32)
    nc.sync.dma_start(out=bdc, in_=bdd[:, None])
    id80 = const.tile([C, C], f32)
    nc.sync.dma_start(out=id80, in_=ident)
    iota80 = const.tile([C, 1], f32)
    nc.sync.dma_start(out=iota80, in_=iota80d)
    iotalab = const.tile([P, C], f32)
    nc.sync.dma_start(out=iotalab, in_=iotalabd)
    labT = const.tile([P, B // P], f32)
    nc.sync.dma_start(out=labT, in_=labelsT)

    # forget-gate bias for layer 2: +1.0 on gate dims [512, 768) = cols 4,5
    nc.vector.tensor_scalar_add(b2c[:, 4:6], b2c[:, 4:6], 1.0)

    # ---------------- E1' = emb @ W1x + b1 (+1 on f-range) ----------------
    startup = tc.alloc_tile_pool(name="startup", bufs=1)
    embT = startup.tile([E, C], DT)
    nc.sync.dma_start(out=embT, in_=emb.rearrange("c e -> e c"))
    W1x = startup.tile([E, G], DT)
    nc.sync.dma_start(out=W1x, in_=W1d[0:E, :])
    b1row = startup.tile([1, G], DT)
    nc.sync.dma_start(out=b1row, in_=b1d[None, :])
    ones1 = startup.tile([1, C], DT)
    nc.vector.memset(ones1, 1.0)

    pstart = tc.alloc_tile_pool(name="pstart", bufs=1, space="PSUM")
    e1ps = pstart.tile([C, G], f32)
    for n in range(2):
        s = slice(512 * n, 512 * (n + 1))
        nc.tensor.matmul(e1ps[:, s], embT, W1x[:, s], start=True, stop=False)
        nc.tensor.matmul(e1ps[:, s], ones1, b1row[:, s], start=False, stop=True)
    E1 = const.tile([C, G], DT)
    nc.scalar.activation(E1, e1ps, AF.Identity, scale=WS)  # pre-scaled like W1h8
    nc.vector.tensor_scalar_add(E1[:, 512:768], E1[:, 512:768], WS)  # forget bias
    pstart.release()
    startup.release()

    # ---------------- pools for the recurrent loop ----------------
    states = tc.alloc_tile_pool(name="states", bufs=2)
    gates = tc.alloc_tile_pool(name="gates", bufs=2)
    pgate = tc.alloc_tile_pool(name="pgate", bufs=2, space="PSUM")
    feats = tc.alloc_tile_pool(name="feats", bufs=3)

    h1 = c1 = h2 = c2 = None
    SI = 1.0 / WS

    def l1_block(t, oh, h1_in):
        # One PSUM tile + ONE activation per gate ([P, 2B] = 4 banks).
        # Emitted in gate PAIRS with all (dependency-free) E1 matmuls first,
        # so the in-order PE queue can pre-run them while h1 is still being
        # produced, instead of stalling at the first DoubleRow matmul.
        h1k = h1_in.rearrange("p (k b) -> p k b", k=2) if t > 0 else None
        e1_only = (t == 0 or thin == "pe")
        sg1 = []
        for gp in range(2):
            pair = (2 * gp, 2 * gp + 1)
            ps_t = {}
            for g in pair:
                ps_t[g] = pgate.tile([P, 2 * B], f32, tag="g", name=f"ps1_{t}_{g}")
                for ml in range(2):
                    m = 2 * g + ml
                    for n in range(NB):
                        dst = ps_t[g][:, B * ml + 512 * n: B * ml + 512 * (n + 1)]
                        rhs_oh = oh[:, 512 * n: 512 * (n + 1)]
                        nc.tensor.matmul(dst, E1[:, P * m: P * (m + 1)], rhs_oh,
                                         start=True, stop=e1_only)
            for g in pair:
                if not e1_only:
                    for ml in range(2):
                        m = 2 * g + ml
                        for n in range(NB):
                            dst = ps_t[g][:, B * ml + 512 * n: B * ml + 512 * (n + 1)]
                            nc.tensor.matmul(
                                dst,
                                W1h8k[:, :, P * m: P * (m + 1)],
                                h1k[:, :, 512 * n: 512 * (n + 1)],
                                start=False, stop=True,
                                perf_mode=mybir.MatmulPerfMode.DoubleRow)
                sg = gates.tile([P, 2 * B], DT, tag=f"sg_{g}", name=f"sg1_{g}")
                act(sg, ps_t[g], GATE_FUNCS[g], scale=SI)
                sg1.append(sg)
        return sg1

    def l2_block(t, h1_in, h2_in):
        h1k = h1_in.rearrange("p (k b) -> p k b", k=2)
        h2k = h2_in.rearrange("p (k b) -> p k b", k=2) if t > 0 else None
        kks = [0] if (t == 0 or thin == "pe") else [0, 1]
        sg2 = []
        for g in range(4):
            sg = gates.tile([P, 2 * B], DT, tag=f"sg_{g}", name=f"sg2_{g}")
            ps = pgate.tile([P, 2 * B], f32, tag="g", name=f"ps2_{t}_{g}")
            for ml in range(2):
                m = 2 * g + ml
                # kk outer so consecutive matmuls share the stationary
                # operand (one LDWEIGHTS per weight tile, not per matmul)
                for ki, kk in enumerate(kks):
                    hsrck = h1k if kk == 0 else h2k
                    for n in range(NB):
                        dst = ps[:, B * ml + 512 * n: B * ml + 512 * (n + 1)]
                        nc.tensor.matmul(
                            dst,
                            W28k[:, 2 * kk: 2 * kk + 2, P * m: P * (m + 1)],
                            hsrck[:, :, 512 * n: 512 * (n + 1)],
                            start=(ki == 0), stop=(ki == len(kks) - 1),
                            perf_mode=mybir.MatmulPerfMode.DoubleRow)
                if not zero_b2:
                    # per-m bias differs across the halves -> act per half
                    act(sg[:, ml * B:(ml + 1) * B], ps[:, ml * B:(ml + 1) * B],
                        GATE_FUNCS[g], bias=b2c[:, m:m + 1], scale=SI)
            if zero_b2:
                # b2 == 0: the only bias is the uniform forget bias (+1.0 on
                # gate f), which fits the float-immediate form -> ONE
                # activation per gate over the whole [P, 2B] tile.
                act(sg, ps, GATE_FUNCS[g],
                    bias=(1.0 if g == 2 else 0.0), scale=SI)
            sg2.append(sg)
        return sg2

    def cell(t, sgates, c_in, ctag, htag, thname, hdt):
        si, sj, sf, so = sgates
        tt(si, si, sj, OP.mult)                  # si <- sigmoid(i)*tanh(j)
        cn = states.tile([P, 2 * B], DT, tag=ctag, name=ctag)
        if t == 0:
            nc.vector.tensor_copy(cn, si)
        else:
            tt(sf, c_in, sf, OP.mult)            # sf <- c * sigmoid(f+1)
            tt(cn, sf, si, OP.add)
        th = gates.tile([P, 2 * B], DT, tag="th", name=thname, bufs=2)
        act(th, cn, AF.Tanh)
        hn = states.tile([P, 2 * B], hdt, tag=htag, name=htag)
        tt(hn, th, so, OP.mult)
        return cn, hn

    # Software pipeline: L2 runs one step behind L1.  The L1 cell (which
    # produces h1(t), the recurrent critical path) is emitted BEFORE the L2
    # block so its tanh isn't queued behind 8 unrelated L2 activations on
    # the in-order ACT engine.
    sg2_pend = None
    n_feat = featrep.shape[0]
    for t in range(T_steps):
        featbc = feats.tile([C, B], DT, tag="featbc")
        nc.sync.dma_start(out=featbc, in_=featrep[t % n_feat])
        oh = feats.tile([C, B], DT, tag="oh")
        nc.vector.tensor_scalar(oh, featbc, iota80[:, 0:1], None, op0=OP.is_equal)

        h1_prev = h1
        sg1 = l1_block(t, oh, h1_prev)
        c1, h1 = cell(t, sg1, c1, "c1", "h1", "th1", FP8)
        if t > 0:
            sg2_pend = l2_block(t - 1, h1_prev, h2)
            c2, h2 = cell(t - 1, sg2_pend, c2, "c2", "h2", "th2", FP8)

    # drain the pipeline: L2 for the final step
    sg2_pend = l2_block(T_steps - 1, h1, h2)
    c2, h2 = cell(T_steps - 1, sg2_pend, c2, "c2", "h2", "th2", FP8)

    feats.release()
    pgate.release()

    # ---------------- loss ----------------
    ploss = tc.alloc_tile_pool(name="ploss", bufs=1, space="PSUM")
    lpool = tc.alloc_tile_pool(name="lpool", bufs=2)

    lps = ploss.tile([C, B], f32, tag="logits")
    h2k = h2.rearrange("p (k b) -> p k b", k=2)
    for n in range(NB):
        nc.tensor.matmul(
            lps[:, 512 * n: 512 * (n + 1)],
            Wd8k,
            h2k[:, :, 512 * n: 512 * (n + 1)],
            start=True, stop=True,
            perf_mode=mybir.MatmulPerfMode.DoubleRow)
    logits = lpool.tile([C, B], f32, tag="logits_sb", bufs=1)
    nc.scalar.activation(logits, lps, AF.Identity, bias=bdc[:, 0:1], scale=SI)

    loss_sb = lpool.tile([P, B // P], f32, tag="loss_sb", bufs=1)
    # per-chunk sum-exps / label logits gathered as COLUMNS of shared tiles so
    # the log and the final subtract are single ops (one Exp->Ln table switch)
    sumexp_all = lpool.tile([P, B // P], f32, tag="sumexp_all", bufs=1)
    lablog_all = lpool.tile([P, B // P], f32, tag="lablog_all", bufs=1)
    for cb in range(B // P):
        lt = ploss.tile([P, C], f32, tag="lt", bufs=2, name=f"lt_{cb}")
        nc.tensor.transpose(lt, logits[:, P * cb: P * (cb + 1)], id80)
        ohl = lpool.tile([P, C], f32, tag="ohl", name=f"ohl_{cb}")
        nc.vector.tensor_scalar(ohl, iotalab, labT[:, cb:cb + 1], None,
                                op0=OP.is_equal)
        scr1 = lpool.tile([P, C], f32, tag="scr1", name=f"scr1_{cb}")
        nc.vector.scalar_tensor_tensor(scr1, lt, 1.0, ohl,
                                       op0=OP.mult, op1=OP.mult,
                                       accum_out=lablog_all[:, cb:cb + 1])
        scr2 = lpool.tile([P, C], f32, tag="scr2", name=f"scr2_{cb}")
        nc.scalar.activation(scr2, lt, AF.Exp,
                             accum_out=sumexp_all[:, cb:cb + 1])
    lse = lpool.tile([P, B // P], f32, tag="lse", bufs=1)
    nc.scalar.activation(lse, sumexp_all, AF.Ln)
    nc.vector.tensor_sub(loss_sb, lse, lablog_all)
    nc.sync.dma_start(out=lossd, in_=loss_sb)
    lpool.release()
    ploss.release()
    gates.release()
    states.release()
    const.release()


# ---------------------------------------------------------------------------
# host side
# ---------------------------------------------------------------------------
_CACHE = {}


def _get_program(zero_b2=False):
    key = ("nc", bool(zero_b2))
    if key not in _CACHE:
        _CACHE[key] = build_program(zero_b2=bool(zero_b2))
    return _CACHE[key]


def make_in_maps(features, labels, embedding, W1, b1, W2, b2, Wd, bd,
                 T_steps=T):
    """Shard the full inputs into 8 per-core input maps (2 packed blobs)."""
    features = np.asarray(features, dtype=np.int32)
    labels = np.asarray(labels, dtype=np.int32)
    import ml_dtypes
    wdt = ml_dtypes.bfloat16 if DT == BF16 else np.float32
    shared16 = [
        np.asarray(embedding, np.float32).astype(wdt).ravel(),
        np.asarray(W1, np.float32).astype(wdt).ravel(),
        np.asarray(b1, np.float32).astype(wdt).ravel(),
        np.asarray(W2, np.float32).astype(wdt).ravel(),
        np.asarray(Wd, np.float32).astype(wdt).ravel(),
    ]
    shared32_tail = [
        np.asarray(b2, np.float32).ravel(),
        np.asarray(bd, np.float32).ravel(),
        np.eye(C, dtype=np.float32).ravel(),
        np.arange(C, dtype=np.float32).ravel(),
        np.ascontiguousarray(
            np.broadcast_to(np.arange(C, dtype=np.float32)[None, :], (P, C))).ravel(),
    ]
    in_maps = []
    for c in range(N_CORES):
        fs = features[B * c: B * (c + 1)]            # [B, T]
        ls = labels[B * c: B * (c + 1)]              # [B]
        ft = fs.T[:T_steps]                          # [T_steps, B]
        featrep = np.ascontiguousarray(
            np.broadcast_to(ft[:, None, :], (T_steps, C, B))).astype(wdt)
        labT = np.ascontiguousarray(ls.reshape(B // P, P).T.astype(np.float32))
        blob16 = np.concatenate([featrep.ravel()] + shared16)
        blob32 = np.concatenate([labT.ravel()] + shared32_tail)
        in_maps.append({"blob16": blob16, "blob32": blob32})
    return in_maps


def gather_output(results):
    outs = []
    for r in results:
        outs.append(np.asarray(r["loss"]).T.reshape(-1))   # [P, B//P] -> [B]
    return np.concatenate(outs, axis=0).astype(np.float32)


def kernel(features, labels, embedding, W1, b1, W2, b2, Wd, bd):
    # b2 == 0 (always true for this model's setup_inputs) enables merged L2
    # activations with float biases; any nonzero b2 uses the general path.
    zb = bool(np.all(np.asarray(b2) == 0))
    nc = _get_program(zero_b2=zb)
    in_maps = make_in_maps(features, labels, embedding, W1, b1, W2, b2, Wd, bd)
    res = run_bass_kernel_spmd(nc, in_maps, core_ids=list(range(N_CORES)))
    return gather_output(res.results)



# revision 36
# speedup vs baseline: 1.4348x; 1.0053x over previous
"""Trainium2 Bass kernel: 2-layer LSTM (T=80, H=256) + embedding + softmax CE loss.

Strategy: data-parallel over batch (8192 -> 8 cores x 1024).  Everything runs
in a transposed layout: states/gates keep the hidden/gate dim on SBUF
partitions and the batch dim on the free axis, so the recurrent matmuls need
no per-step transposes (stationary = weights, moving = state).

The embedding lookup is reformulated as a one-hot matmul:
    x_t @ W1x  ==  onehot(feat_t) @ (emb @ W1x) = onehot @ E1
E1' = WS*(emb @ W1x + b1 + forget_bias_one_hot_fold), so layer-1 needs no
bias adds at all.  The one-hot itself is an is_equal compare against an iota
column, with the feature row replicated across 80 partitions host-side.

A logical [256, 1024] tensor is stored "folded" as one SBUF tile [128, 2048]:
hidden unit u lives at (partition u % 128, col-block u // 128).

Perf structure (HW-validated at ~21.5 us/step, ACT-engine-bound):
- Recurrent weights + h states are fp8e4 scaled by WS=8; the K=256 (L1) and
  K=512 (L2) contractions run as DoubleRow matmuls (2 fp8 weights/PE cell ->
  half the instructions, 0.5 cyc/row).  Gate activations apply scale=1/WS.
  The folded [128, 2, B] state layout is exactly DoubleRow's 3D-AP shape.
- Per gate: ONE [128, 2B] PSUM tile (4 banks) + ONE activation.  When b2 is
  nonzero the L2 acts fall back to per-half form (the act bias operand is
  per-partition [P,1] and the halves need different b2 slices); kernel()
  picks the fast path at runtime since this model's b2 is all zeros (only
  the uniform forget bias, which fits a float immediate).
- Iteration emits l1_block -> L1 cell -> l2_block -> L2 cell (L2 one step
  behind), so the tanh on the h1 critical path is not queued behind 8
  unrelated L2 activations on the in-order ACT engine; L1 emits its
  dependency-free E1 matmuls for a gate PAIR before the h1-dependent
  DoubleRow matmuls so the in-order PE queue pre-runs them.
- All 13 logical inputs are packed into 2 flat DRAM blobs (bf16 + f32):
  per-call dispatch overhead is ~64us PER INPUT BUFFER (~0.8ms for 13).

Final loss: logits via one DoubleRow matmul Wd.T @ h2 -> [80, B],
PE-transposed back to [B-chunk, 80] so log-sum-exp and the label gather run
along the free axis.
"""

import sys

sys.path.insert(0, "/opt/trn_rl_repo")

import numpy as np

import concourse.bass as bass
import concourse.mybir as mybir
import concourse.tile as tile
from concourse import bacc
from concourse.bass_utils import run_bass_kernel_spmd

AF = mybir.ActivationFunctionType
OP = mybir.AluOpType
F32 = mybir.dt.float32
BF16 = mybir.dt.bfloat16
I32 = mybir.dt.int32
FP8 = mybir.dt.float8e4
DT = BF16          # dtype for weights / states / gate activations (matmul operands)
WS = 8.0           # fp8 weight pre-scale (undone by activation scale=1/WS)

P = 128          # partitions
N_CORES = 8
B = 1024         # per-core batch shard
T = 80           # seq len
C = 80           # num classes
E = 8            # emb dim
H = 256          # hidden
G = 4 * H        # gates = 1024
NB = B // 512    # moving-operand chunks of 512 (fp32 max free dim)

GATE_FUNCS = [AF.Sigmoid, AF.Tanh, AF.Sigmoid, AF.Sigmoid]  # i, j, f, o


def _blob16_offsets(FT):
    sizes = [("featrep", FT * C * B), ("emb", C * E), ("W1", (E + H) * G),
             ("b1", G), ("W2", 2 * H * G), ("Wd", H * C)]
    out, o = {}, 0
    for k, ln in sizes:
        out[k] = (o, ln)
        o += ln
    out["end"] = o
    return out


def _blob32_offsets():
    sizes = [("labelsT", P * (B // P)), ("b2", G), ("bd", C),
             ("ident80", C * C), ("iota80", C), ("iotalab", P * C)]
    out, o = {}, 0
    for k, ln in sizes:
        out[k] = (o, ln)
        o += ln
    out["end"] = o
    return out


def build_program(T_steps=T, thin=None, featrep_steps=None, zero_b2=False):
    # Bacc (not plain Bass): its compile() runs generate_event_semaphores,
    # which splits excess per-instruction sync waits onto InstEventSemaphore
    # nops — walrus only allows one wait on LDWEIGHTS/MATMULT.
    nc = bacc.Bacc("TRN2", target_bir_lowering=False, debug=False,
                   enable_asserts=False, num_devices=N_CORES)

    # ---------------- DRAM I/O ----------------
    # All inputs are packed into TWO flat buffers (one bf16, one f32): the
    # per-call PJRT/axon dispatch overhead is ~64us PER BUFFER, so 13 separate
    # inputs cost ~0.8ms/call in pure launch overhead.
    FT = featrep_steps or T_steps
    off16 = _blob16_offsets(FT)
    off32 = _blob32_offsets()
    blob16 = nc.dram_tensor("blob16", [off16["end"]], DT, kind="ExternalInput").ap()
    blob32 = nc.dram_tensor("blob32", [off32["end"]], F32, kind="ExternalInput").ap()

    def v16(key, shape_str=None, **axes):
        o, ln = off16[key]
        v = blob16[o:o + ln]
        return v.rearrange(shape_str, **axes) if shape_str else v

    def v32(key, shape_str=None, **axes):
        o, ln = off32[key]
        v = blob32[o:o + ln]
        return v.rearrange(shape_str, **axes) if shape_str else v

    featrep = v16("featrep", "(t c b) -> t c b", c=C, b=B)
    emb = v16("emb", "(c e) -> c e", e=E)
    W1d = v16("W1", "(r g) -> r g", g=G)
    b1d = v16("b1")
    W2d = v16("W2", "(r g) -> r g", g=G)
    Wdd = v16("Wd", "(r c) -> r c", c=C)
    labelsT = v32("labelsT", "(p m) -> p m", m=B // P)
    b2d = v32("b2")
    bdd = v32("bd")
    ident = v32("ident80", "(a b) -> a b", b=C)
    iota80d = v32("iota80", "(a b) -> a b", b=1)
    iotalabd = v32("iotalab", "(p c) -> p c", c=C)
    lossd = nc.dram_tensor("loss", [P, B // P], F32, kind="ExternalOutput").ap()

    with tile.TileContext(nc) as tc:
        _emit(nc, tc, featrep, labelsT, emb, W1d, b1d, W2d, b2d, Wdd, bdd,
              ident, iota80d, iotalabd, lossd, T_steps, thin, zero_b2)
    nc.compile()
    return nc


def _emit(nc, tc, featrep, labelsT, emb, W1d, b1d, W2d, b2d, Wdd, bdd,
          ident, iota80d, iotalabd, lossd, T_steps=T, thin=None,
          zero_b2=False):
    f32, i32 = F32, I32

    def act(out, in_, func, **kw):
        if thin == "act":
            nc.scalar.activation(out[:, 0:32], in_[:, 0:32], func, **kw)
        else:
            nc.scalar.activation(out, in_, func, **kw)

    def tt(out, a, b_, op):
        if thin == "dve":
            nc.vector.tensor_tensor(out[:, 0:32], a[:, 0:32], b_[:, 0:32], op=op)
        else:
            nc.vector.tensor_tensor(out, a, b_, op=op)

    const = tc.alloc_tile_pool(name="const", bufs=1)

    # ---------------- resident weights/constants ----------------
    # Recurrent weights are stored fp8e4 scaled by WS; the gate activations
    # apply scale=1/WS to undo it.  fp8 + DoubleRow runs the K=256 contraction
    # in ONE matmul at 0.5 cycles/row (2x PE throughput, half the instructions).
    W1h = const.tile([P, 2 * G], DT)          # k-chunk k at cols [G k, G k + G)
    for k in range(2):
        nc.sync.dma_start(out=W1h[:, k * G:(k + 1) * G],
                          in_=W1d[E + P * k: E + P * (k + 1), :])
    W2 = const.tile([P, 4 * G], DT)
    for k in range(4):
        nc.sync.dma_start(out=W2[:, k * G:(k + 1) * G],
                          in_=W2d[P * k: P * (k + 1), :])
    Wd = const.tile([P, 2 * C], DT)
    for k in range(2):
        nc.sync.dma_start(out=Wd[:, k * C:(k + 1) * C],
                          in_=Wdd[P * k: P * (k + 1), :])
    W1h8 = const.tile([P, 2 * G], FP8)
    nc.scalar.activation(W1h8, W1h, AF.Identity, scale=WS)
    W28 = const.tile([P, 4 * G], FP8)
    nc.scalar.activation(W28, W2, AF.Identity, scale=WS)
    Wd8 = const.tile([P, 2 * C], FP8)
    nc.scalar.activation(Wd8, Wd, AF.Identity, scale=WS)
    W1h8k = W1h8.rearrange("p (k g) -> p k g", k=2)
    W28k = W28.rearrange("p (k g) -> p k g", k=4)
    Wd8k = Wd8.rearrange("p (k c) -> p k c", k=2)
    b2c = const.tile([P, G // P], f32)          # col m = b2[128 m : 128 m + 128]
    nc.sync.dma_start(out=b2c, in_=b2d.rearrange("(m p) -> p m", p=P))

    bdc = const.tile([C, 1], f32)
    nc.sync.dma_start(out=bdc, in_=bdd[:, None])
    id80 = const.tile([C, C], f32)
    nc.sync.dma_start(out=id80, in_=ident)
    iota80 = const.tile([C, 1], f32)
    nc.sync.dma_start(out=iota80, in_=iota80d)
    iotalab = const.tile([P, C], f32)
    nc.sync.dma_start(out=iotalab, in_=iotalabd)
    labT = const.tile([P, B // P], f32)
    nc.sync.dma_start(out=labT, in_=labelsT)

    # forget-gate bias for layer 2: +1.0 on gate dims [512, 768) = cols 4,5
    nc.vector.tensor_scalar_add(b2c[:, 4:6], b2c[:, 4:6], 1.0)

    # ---------------- E1' = emb @ W1x + b1 (+1 on f-range) ----------------
    startup = tc.alloc_tile_pool(name="startup", bufs=1)
    embT = startup.tile([E, C], DT)
    nc.sync.dma_start(out=embT, in_=emb.rearrange("c e -> e c"))
    W1x = startup.tile([E, G], DT)
    nc.sync.dma_start(out=W1x, in_=W1d[0:E, :])
    b1row = startup.tile([1, G], DT)
    nc.sync.dma_start(out=b1row, in_=b1d[None, :])
    ones1 = startup.tile([1, C], DT)
    nc.vector.memset(ones1, 1.0)

    pstart = tc.alloc_tile_pool(name="pstart", bufs=1, space="PSUM")
    e1ps = pstart.tile([C, G], f32)
    for n in range(2):
        s = slice(512 * n, 512 * (n + 1))
        nc.tensor.matmul(e1ps[:, s], embT, W1x[:, s], start=True, stop=False)
        nc.tensor.matmul(e1ps[:, s], ones1, b1row[:, s], start=False, stop=True)
    E1 = const.tile([C, G], DT)
    nc.scalar.activation(E1, e1ps, AF.Identity, scale=WS)  # pre-scaled like W1h8
    nc.vector.tensor_scalar_add(E1[:, 512:768], E1[:, 512:768], WS)  # forget bias
    pstart.release()
    startup.release()

    # ---------------- pools for the recurrent loop ----------------
    states = tc.alloc_tile_pool(name="states", bufs=2)
    gates = tc.alloc_tile_pool(name="gates", bufs=2)
    pgate = tc.alloc_tile_pool(name="pgate", bufs=2, space="PSUM")
    feats = tc.alloc_tile_pool(name="feats", bufs=3)

    h1 = c1 = h2 = c2 = None
    SI = 1.0 / WS

    def l1_block(t, oh, h1_in):
        # One PSUM tile + ONE activation per gate ([P, 2B] = 4 banks).
        # Emitted in gate PAIRS with all (dependency-free) E1 matmuls first,
        # so the in-order PE queue can pre-run them while h1 is still being
        # produced, instead of stalling at the first DoubleRow matmul.
        h1k = h1_in.rearrange("p (k b) -> p k b", k=2) if t > 0 else None
        e1_only = (t == 0 or thin == "pe")
        sg1 = []
        for gp in range(2):
            pair = (2 * gp, 2 * gp + 1)
            ps_t = {}
            for g in pair:
                ps_t[g] = pgate.tile([P, 2 * B], f32, tag="g", name=f"ps1_{t}_{g}")
                for ml in range(2):
                    m = 2 * g + ml
                    for n in range(NB):
                        dst = ps_t[g][:, B * ml + 512 * n: B * ml + 512 * (n + 1)]
                        rhs_oh = oh[:, 512 * n: 512 * (n + 1)]
                        nc.tensor.matmul(dst, E1[:, P * m: P * (m + 1)], rhs_oh,
                                         start=True, stop=e1_only)
            for g in pair:
                if not e1_only:
                    for ml in range(2):
                        m = 2 * g + ml
                        for n in range(NB):
                            dst = ps_t[g][:, B * ml + 512 * n: B * ml + 512 * (n + 1)]
                            nc.tensor.matmul(
                                dst,
                                W1h8k[:, :, P * m: P * (m + 1)],
                                h1k[:, :, 512 * n: 512 * (n + 1)],
                                start=False, stop=True,
                                perf_mode=mybir.MatmulPerfMode.DoubleRow)
                sg = gates.tile([P, 2 * B], DT, tag=f"sg_{g}", name=f"sg1_{g}")
                act(sg, ps_t[g], GATE_FUNCS[g], scale=SI)
                sg1.append(sg)
        return sg1

    def l2_block(t, h1_in, h2_in):
        h1k = h1_in.rearrange("p (k b) -> p k b", k=2)
        h2k = h2_in.rearrange("p (k b) -> p k b", k=2) if t > 0 else None
        kks = [0] if (t == 0 or thin == "pe") else [0, 1]
        sg2 = []
        for g in range(4):
            sg = gates.tile([P, 2 * B], DT, tag=f"sg_{g}", name=f"sg2_{g}")
            ps = pgate.tile([P, 2 * B], f32, tag="g", name=f"ps2_{t}_{g}")
            for ml in range(2):
                m = 2 * g + ml
                # kk outer so consecutive matmuls share the stationary
                # operand (one LDWEIGHTS per weight tile, not per matmul)
                for ki, kk in enumerate(kks):
                    hsrck = h1k if kk == 0 else h2k
                    for n in range(NB):
                        dst = ps[:, B * ml + 512 * n: B * ml + 512 * (n + 1)]
                        nc.tensor.matmul(
                            dst,
                            W28k[:, 2 * kk: 2 * kk + 2, P * m: P * (m + 1)],
                            hsrck[:, :, 512 * n: 512 * (n + 1)],
                            start=(ki == 0), stop=(ki == len(kks) - 1),
                            perf_mode=mybir.MatmulPerfMode.DoubleRow)
                if not zero_b2:
                    # per-m bias differs across the halves -> act per half
                    act(sg[:, ml * B:(ml + 1) * B], ps[:, ml * B:(ml + 1) * B],
                        GATE_FUNCS[g], bias=b2c[:, m:m + 1], scale=SI)
            if zero_b2:
                # b2 == 0: the only bias is the uniform forget bias (+1.0 on
                # gate f), which fits the float-immediate form -> ONE
                # activation per gate over the whole [P, 2B] tile.
                act(sg, ps, GATE_FUNCS[g],
                    bias=(1.0 if g == 2 else 0.0), scale=SI)
            sg2.append(sg)
        return sg2

    def cell(t, sgates, c_in, ctag, htag, thname, hdt):
        si, sj, sf, so = sgates
        tt(si, si, sj, OP.mult)                  # si <- sigmoid(i)*tanh(j)
        cn = states.tile([P, 2 * B], DT, tag=ctag, name=ctag)
        if t == 0:
            nc.vector.tensor_copy(cn, si)
        else:
            tt(sf, c_in, sf, OP.mult)            # sf <- c * sigmoid(f+1)
            tt(cn, sf, si, OP.add)
        th = gates.tile([P, 2 * B], DT, tag="th", name=thname, bufs=2)
        act(th, cn, AF.Tanh)
        hn = states.tile([P, 2 * B], hdt, tag=htag, name=htag)
        tt(hn, th, so, OP.mult)
        return cn, hn

    # Software pipeline: L2 runs one step behind L1.  The L1 cell (which
    # produces h1(t), the recurrent critical path) is emitted BEFORE the L2
    # block so its tanh isn't queued behind 8 unrelated L2 activations on
    # the in-order ACT engine.
    sg2_pend = None
    n_feat = featrep.shape[0]
    for t in range(T_steps):
        featbc = feats.tile([C, B], DT, tag="featbc")
        nc.sync.dma_start(out=featbc, in_=featrep[t % n_feat])
        oh = feats.tile([C, B], DT, tag="oh")
        nc.vector.tensor_scalar(oh, featbc, iota80[:, 0:1], None, op0=OP.is_equal)

        h1_prev = h1
        sg1 = l1_block(t, oh, h1_prev)
        c1, h1 = cell(t, sg1, c1, "c1", "h1", "th1", FP8)
        if t > 0:
            sg2_pend = l2_block(t - 1, h1_prev, h2)
            c2, h2 = cell(t - 1, sg2_pend, c2, "c2", "h2", "th2", FP8)

    # drain the pipeline: L2 for the final step
    sg2_pend = l2_block(T_steps - 1, h1, h2)
    c2, h2 = cell(T_steps - 1, sg2_pend, c2, "c2", "h2", "th2", FP8)

    feats.release()
    pgate.release()

    # ---------------- loss ----------------
    ploss = tc.alloc_tile_pool(name="ploss", bufs=1, space="PSUM")
    lpool = tc.alloc_tile_pool(name="lpool", bufs=2)

    lps = ploss.tile([C, B], f32, tag="logits")
    h2k = h2.rearrange("p (k b) -> p k b", k=2)
    for n in range(NB):
        nc.tensor.matmul(
            lps[:, 512 * n: 512 * (n + 1)],
            Wd8k,
            h2k[:, :, 512 * n: 512 * (n + 1)],
            start=True, stop=True,
            perf_mode=mybir.MatmulPerfMode.DoubleRow)
    logits = lpool.tile([C, B], f32, tag="logits_sb", bufs=1)
    nc.scalar.activation(logits, lps, AF.Identity, bias=bdc[:, 0:1], scale=SI)

    loss_sb = lpool.tile([P, B // P], f32, tag="loss_sb", bufs=1)
    # per-chunk sum-exps / label logits gathered as COLUMNS of shared tiles so
    # the log and the final subtract are single ops (one Exp->Ln table switch)
    sumexp_all = lpool.tile([P, B // P], f32, tag="sumexp_all", bufs=1)
    lablog_all = lpool.tile([P, B // P], f32, tag="lablog_all", bufs=1)
    for cb in range(B // P):
        lt = ploss.tile([P, C], f32, tag="lt", bufs=2, name=f"lt_{cb}")
        nc.tensor.transpose(lt, logits[:, P * cb: P * (cb + 1)], id80)
        ohl = lpool.tile([P, C], f32, tag="ohl", name=f"ohl_{cb}")
        nc.vector.tensor_scalar(ohl, iotalab, labT[:, cb:cb + 1], None,
                                op0=OP.is_equal)
        scr1 = lpool.tile([P, C], f32, tag="scr1", name=f"scr1_{cb}")
        nc.vector.scalar_tensor_tensor(scr1, lt, 1.0, ohl,
                                       op0=OP.mult, op1=OP.mult,
                                       accum_out=lablog_all[:, cb:cb + 1])
        scr2 = lpool.tile([P, C], f32, tag="scr2", name=f"scr2_{cb}")
        nc.scalar.activation(scr2, lt, AF.Exp,
                             accum_out=sumexp_all[:, cb:cb + 1])
    lse = lpool.tile([P, B // P], f32, tag="lse", bufs=1)
    nc.scalar.activation(lse, sumexp_all, AF.Ln)
    nc.vector.tensor_sub(loss_sb, lse, lablog_all)
    nc.sync.dma_start(out=lossd, in_=loss_sb)
    lpool.release()
    ploss.release()
    gates.release()
    states.release()
    const.release()


# ---------------------------------------------------------------------------
# host side
# ---------------------------------------------------------------------------
_CACHE = {}


def _get_program(zero_b2=False):
    key = ("nc", bool(zero_b2))
    if key not in _CACHE:
        _CACHE[key] = build_program(zero_b2=bool(zero_b2))
    return _CACHE[key]


def make_in_maps(features, labels, embedding, W1, b1, W2, b2, Wd, bd,
                 T_steps=T):
    """Shard the full inputs into 8 per-core input maps (2 packed blobs)."""
    features = np.asarray(features, dtype=np.int32)
    labels = np.asarray(labels, dtype=np.int32)
    import ml_dtypes
    wdt = ml_dtypes.bfloat16 if DT == BF16 else np.float32
    shared16 = [
        np.asarray(embedding, np.float32).astype(wdt).ravel(),
        np.asarray(W1, np.float32).astype(wdt).ravel(),
        np.asarray(b1, np.float32).astype(wdt).ravel(),
        np.asarray(W2, np.float32).astype(wdt).ravel(),
        np.asarray(Wd, np.float32).astype(wdt).ravel(),
    ]
    shared32_tail = [
        np.asarray(b2, np.float32).ravel(),
        np.asarray(bd, np.float32).ravel(),
        np.eye(C, dtype=np.float32).ravel(),
        np.arange(C, dtype=np.float32).ravel(),
        np.ascontiguousarray(
            np.broadcast_to(np.arange(C, dtype=np.float32)[None, :], (P, C))).ravel(),
    ]
    in_maps = []
    for c in range(N_CORES):
        fs = features[B * c: B * (c + 1)]            # [B, T]
        ls = labels[B * c: B * (c + 1)]              # [B]
        ft = fs.T[:T_steps]                          # [T_steps, B]
        featrep = np.ascontiguousarray(
            np.broadcast_to(ft[:, None, :], (T_steps, C, B))).astype(wdt)
        labT = np.ascontiguousarray(ls.reshape(B // P, P).T.astype(np.float32))
        blob16 = np.concatenate([featrep.ravel()] + shared16)
        blob32 = np.concatenate([labT.ravel()] + shared32_tail)
        in_maps.append({"blob16": blob16, "blob32": blob32})
    return in_maps


def gather_output(results):
    outs = []
    for r in results:
        outs.append(np.asarray(r["loss"]).T.reshape(-1))   # [P, B//P] -> [B]
    return np.concatenate(outs, axis=0).astype(np.float32)


def kernel(features, labels, embedding, W1, b1, W2, b2, Wd, bd):
    # b2 == 0 (always true for this model's setup_inputs) enables merged L2
    # activations with float biases; any nonzero b2 uses the general path.
    zb = bool(np.all(np.asarray(b2) == 0))
    nc = _get_program(zero_b2=zb)
    in_maps = make_in_maps(features, labels, embedding, W1, b1, W2, b2, Wd, bd)
    res = run_bass_kernel_spmd(nc, in_maps, core_ids=list(range(N_CORES)))
    return gather_output(res.results)

